# revision 54
# baseline (speedup 1.0000x reference)
"""CollaborativeAttention (complex-valued, per-head mixed queries) on 8 trn2 cores.

Sharding: B*H = 24 (batch, head) units -> 3 heads per core.
  core c: batch b = c // 4, head block hb = c % 4 -> heads [3*hb, 3*hb+2].
Each core computes q/k projections for its batch (replicated within the
4-core batch group), v/cb projections for its head block only, then
scores+softmax+context for its 3 heads.

Karatsuba everywhere: a complex matmul (A_r+iA_i)(B_r+iB_i) is computed as
three real accumulation streams t1=A_r B_r, t2=A_i B_i, t3=(A_r+A_i)(B_r+B_i)
with real = t1-t2, imag = t3-t1-t2 — 25% less PE time than the 4-stream form.

 - Projections: hidden is fully resident in SBUF; hs=hr+hi is built by the
   Pool engine right behind the DMA stream, ws=wr+wi per weight tile by DVE,
   so every HBM byte is fetched exactly once on the (single-slot) DMA path,
   ordered so the matmul chains chase the stream. Combines: s2=copy(t2) on
   Act, then real = t1-s2, x = t3-s2, imag = x-t1 on DVE (one PSUM operand
   per op). kTs = kr+ki (the score Karatsuba stationary) overwrites the hs
   tile on Pool during the v/cb window.
 - Scores run transposed, sT[t, s] (key pos on partitions), in fp32r (full
   PE rate at >=256 moving rows). The exp is FACTORED instead of combined:
     Er = exp((t1-t2+cbr)/8) = exp((t1+cbr)/8) * exp(-t2/8) = e1*e2
     Ei = exp((t3-t1-t2+cbi)/8) = e3*e2*e4,  e4 = exp(-t1/8)
   so the Act engine reads each PSUM bank directly (4 exps, content bias
   fused as a per-partition ACT bias) and DVE does 3 cheap fp16 multiplies;
   no PSUM->SBUF staging pass is needed.
 - mixed queries: mqr/mqi via DVE tensor_scalar ops with per-partition
   mixing scalars; mqs = mqr+mqi on Pool. Double-buffered so the next
   (head, s-slice) block's DVE/Pool work overlaps this block's PE.
 - context matmul (moving dim 129 < 256) runs in fp16: probs and the
   per-head value matrix [vr | vi | 1] (ones column = softmax denominator);
   normalization happens on the tiny [128, 64] context tiles after an Act
   stage-out of PSUM (Pool has no PSUM port; tensor_scalar is not a legal
   Pool opcode — engine placement is ISA-constrained).
This walrus build encodes at most one sync-wait per instruction, so a
post-pass (_split_multi_waits) peels extra waits onto NoOps.
"""

import sys

for _p in ("/opt/trn_rl_repo", "/root/.axon_site", "/root/.axon_site/_ro/trn_rl_repo",
           "/root/.axon_site/_ro/pypackages"):
    if _p not in sys.path:
        sys.path.append(_p)

import numpy as np

import concourse.bass as bass
import concourse.mybir as mybir
import concourse.tile as tile
from concourse.bass_utils import run_bass_kernel_spmd

B, S, D, H = 2, 1024, 768, 12
DK = DV = 768
DH = DV // H          # 64 per-head value dim
HPC = 3               # heads per core
N_CORES = 8
P = 128
ND = D // P           # 6 d-tiles (contraction)
NDK = DK // P         # 6 output n-tiles for q/k
NT = S // P           # 8 token tiles
SWP = 512             # s-half width in projection PSUM tiles
SWS = 256             # s-slice width for scores
NSS = S // SWS        # 4 score s-slices
VC = HPC * DH         # 192 value cols per core
WVCB = 2 * VC + 2 * HPC  # 390: [Wv_r | Wv_i | Wcb_r | Wcb_i] cols

FP = mybir.dt.float32
FR = mybir.dt.float32r
HF = mybir.dt.float16
AF = mybir.ActivationFunctionType
OP = mybir.AluOpType

TRACE = False
LAST_RESULTS = None

_compiled = None


def _split_multi_waits(nc):
    """The walrus build here encodes at most ONE sync-wait per instruction
    ("Too many sync wait commands" in setupSyncWait otherwise). Tile freely
    emits several. Split the extras onto single-wait NoOps that precede the
    instruction in the same engine stream."""
    for fn in nc.m.functions:
        for bb in fn.blocks:
            out = []
            for ins in bb.instructions:
                si = ins.sync_info
                if si is not None and len(si.on_wait) > 1:
                    waits = list(si.on_wait)
                    for j, w in enumerate(waits[:-1]):
                        nop = mybir.InstNoOp(name=f"{ins.name}-ws{j}",
                                             ins=[], outs=[])
                        nop.engine = ins.engine
                        nop.sync_info = mybir.SyncInfo(on_wait=[w], on_update=[])
                        out.append(nop)
                    ins.sync_info = mybir.SyncInfo(on_wait=[waits[-1]],
                                                   on_update=list(si.on_update))
                out.append(ins)
            bb.instructions = out


def _build(split_waits=True):
    """Build the SPMD Bass program (identical on all 8 cores)."""
    nc = bass.Bass(trn_type="TRN2")

    hTr_d = nc.dram_tensor("hTr", [ND, P, S], FR, kind="ExternalInput")
    hTi_d = nc.dram_tensor("hTi", [ND, P, S], FR, kind="ExternalInput")
    wq_d = {c: nc.dram_tensor(f"wq{c}", [ND, P, DK], FR, kind="ExternalInput")
            for c in "ri"}
    wk_d = {c: nc.dram_tensor(f"wk{c}", [ND, P, DK], FR, kind="ExternalInput")
            for c in "ri"}
    wvcb_d = nc.dram_tensor("wvcb", [ND, P, WVCB], FR, kind="ExternalInput")
    bvec_d = nc.dram_tensor("bvec", [1, P + WVCB], FR, kind="ExternalInput")
    mixv_d = nc.dram_tensor("mixv", [HPC, NDK, 3, P], FP, kind="ExternalInput")
    out_d = nc.dram_tensor("out", [2, HPC, NT, P, DH], FP, kind="ExternalOutput")

    with tile.TileContext(nc) as tc:
        with (
            tc.tile_pool(name="persist", bufs=1) as persist,
            tc.tile_pool(name="vsmall", bufs=1) as vsmall,
        ):
            # ---- persistent tensors -------------------------------------
            # (kTs reuses the hs tile below — computed in place at the end
            # of phase P so no extra 24KB/partition is needed)
            qTr = persist.tile([P, NDK, S], FP)
            qTi = persist.tile([P, NDK, S], FP)
            kTr = persist.tile([P, NDK, S], FR)
            kTi = persist.tile([P, NDK, S], FR)

            # [mr | mi | -mi] per (h, a), col = (h*NDK + a)*3 + comp
            # On the Act queue: its SEQ only reaches this issue after real
            # work, keeping the single-slot DMA engine free at t=0 for the
            # weight/hidden loads the first matmuls block on.
            mixv = vsmall.tile([P, HPC * NDK * 3], FP)
            nc.scalar.dma_start(
                mixv, mixv_d[:].rearrange("h a c p -> p (h a c)"))
            # per-head context rhs: [vr_h | vi_h | 1]
            vaug = [vsmall.tile([P, NT, 2 * DH + 1], HF, tag=f"vaug{h}",
                                name=f"vaug{h}")
                    for h in range(HPC)]
            for h in range(HPC):
                nc.vector.memset(vaug[h][:, :, 2 * DH], 1.0)
            # (cbr/8 | cbi/8) per head, flattened: col = tt*2*HPC + (0|HPC) + h
            cb8 = vsmall.tile([P, NT * 2 * HPC], FP)

            # ---- phase P: projections -----------------------------------
            # hs holds hr+hi for the projection Karatsuba t3 streams; once
            # q/k are done it is overwritten with kTs = kr+ki (the score
            # Karatsuba stationary), so its 24KB/partition is reused and the
            # adds hide under the v/cb matmul window.
            ctx_hsp = tc.tile_pool(name="hsp", bufs=1)
            hsp = ctx_hsp.__enter__()
            hs = hsp.tile([P, ND, S], FR, tag="hs")
            with tc.tile_pool(name="hload", bufs=1) as hload:
                hr = hload.tile([P, ND, S], FR, tag="hr")
                hi = hload.tile([P, ND, S], FR, tag="hi")

                def _emit_hidden_half(j):
                    # Half-plane hidden DMAs on the same HWDGE queue as the
                    # weights, emitted between weight loads so the single
                    # DMA slot serves the matmul chains in consumption
                    # order. hs = hr+hi is built by the otherwise-idle Pool
                    # engine right behind the DMA stream instead of being
                    # shipped from host.
                    ssl = slice(j * SWP, (j + 1) * SWP)
                    for d in range(ND):
                        nc.sync.dma_start(hi[:, d, ssl], hTi_d[d, :, ssl])
                        nc.sync.dma_start(hr[:, d, ssl], hTr_d[d, :, ssl])
                    for d in range(ND):
                        nc.gpsimd.tensor_add(hs[:, d, ssl], hr[:, d, ssl],
                                             hi[:, d, ssl])

                # q/k projections: out[n, s] = sum_d W[d, n] * hT[d, s]
                ctx_vw = tc.tile_pool(name="vwides", bufs=1)
                vwides = ctx_vw.__enter__()
                with (
                    tc.tile_pool(name="wload", bufs=2) as wload,
                    tc.tile_pool(name="wsload", bufs=1) as wsload,
                    tc.tile_pool(name="pp1", bufs=2, space="PSUM") as pp1,
                    tc.tile_pool(name="pp23", bufs=1, space="PSUM") as pp23,
                    tc.tile_pool(name="stage", bufs=2) as stage,
                    tc.tile_pool(name="stagex", bufs=1) as stagex,
                ):
                    for (w_d, dst_r, dst_i, gname) in (
                        (wk_d, kTr, kTi, "k"),
                        (wq_d, qTr, qTi, "q"),
                    ):
                        for nt in range(NDK):
                            nsl = slice(nt * P, (nt + 1) * P)
                            t1 = [pp1.tile([P, SWP], FP, tag=f"t1s{j}",
                                           name=f"{gname}t1s{j}n{nt}")
                                  for j in range(2)]
                            t2 = [pp23.tile([P, SWP], FP, tag=f"t2s{j}",
                                            name=f"{gname}t2s{j}n{nt}")
                                  for j in range(2)]
                            t3 = [pp23.tile([P, SWP], FP, tag=f"t3s{j}",
                                            name=f"{gname}t3s{j}n{nt}")
                                  for j in range(2)]
                            wr = wload.tile([P, ND, P], FR, tag="wr")
                            wi = wload.tile([P, ND, P], FR, tag="wi")
                            ws = wsload.tile([P, ND, P], FR, tag="ws")
                            # wi first: the t2 chains consume it first
                            nc.sync.dma_start(
                                wi, w_d["i"][:, :, nsl].rearrange("a p c -> p a c"))
                            nc.sync.dma_start(
                                wr, w_d["r"][:, :, nsl].rearrange("a p c -> p a c"))
                            nc.vector.tensor_add(ws, wr, wi)
                            if gname == "k" and nt == 0:
                                # all hidden DMAs must precede the first
                                # chains in program order (deps are tracked
                                # in program order); nt0's weights went
                                # first on the queue so its chains start
                                # while hidden streams in behind them
                                _emit_hidden_half(0)
                                _emit_hidden_half(1)
                            # t2 chains first so their stops land earliest
                            # and the Act stage copy overlaps the t1/t3 tails
                            for src, tt_, htile in ((wi, t2, hi), (wr, t1, hr),
                                                    (ws, t3, hs)):
                                for j in range(2):
                                    ssl = slice(j * SWP, (j + 1) * SWP)
                                    for d in range(ND):
                                        nc.tensor.matmul(
                                            tt_[j], src[:, d], htile[:, d, ssl],
                                            start=d == 0, stop=d == ND - 1)
                            for j in range(2):
                                ssl = slice(j * SWP, (j + 1) * SWP)
                                s2 = stage.tile([P, SWP], FP, tag="s2")
                                nc.scalar.activation(s2, t2[j], AF.Copy)
                                # real = t1 - t2 ; imag = (t3 - t2) - t1
                                nc.vector.tensor_sub(dst_r[:, nt, ssl], t1[j], s2)
                                x = stagex.tile([P, SWP], FP, tag="x")
                                nc.vector.tensor_sub(x, t3[j], s2)
                                nc.vector.tensor_sub(dst_i[:, nt, ssl], x, t1[j])

                        if gname == "k":
                            # issue the v/cb-pass loads now, on the Act queue
                            # (reached after k's stage copies): clear of the
                            # startup DMA rush, landing long before the v/cb
                            # pass consumes them
                            bvec_sb = vwides.tile([1, P + WVCB], FR)
                            nc.scalar.dma_start(bvec_sb, bvec_d[:])
                            wvcb_sb = vwides.tile([P, ND, WVCB], FR)
                            nc.scalar.dma_start(
                                wvcb_sb, wvcb_d[:].rearrange("a p c -> p a c"))

                    # kTs = kr + ki, overwriting hs in place (program order
                    # is past q's t3 reads here). On the Pool engine so DVE
                    # reaches the first mq block during the v/cb pass.
                    for nt in range(NDK):
                        nc.gpsimd.tensor_add(hs[:, nt], kTr[:, nt], kTi[:, nt])



                # v / cb projections: [tok, c] = sum_d hT[d, tok] * Wbig[d, c]
                with (
                    tc.tile_pool(name="pv", bufs=2, space="PSUM") as pv,
                    tc.tile_pool(name="cbtmp", bufs=2) as cbtmp,
                    tc.tile_pool(name="vstage", bufs=4) as vstage,
                ):
                    for tt in range(NT):
                        tsl = slice(tt * P, (tt + 1) * P)
                        psA = pv.tile([P, WVCB], FP, tag="psA")
                        psB = pv.tile([P, WVCB], FP, tag="psB")
                        # bias row (bv | 0) into psA first (fewest deps first)
                        nc.tensor.matmul(psA, bvec_sb[:, :P], bvec_sb[:, P:],
                                         start=True, stop=False)
                        for d in range(ND):
                            nc.tensor.matmul(psA, hr[:, d, tsl], wvcb_sb[:, d],
                                             start=False, stop=(d == ND - 1))
                            nc.tensor.matmul(psB, hi[:, d, tsl], wvcb_sb[:, d],
                                             start=(d == 0), stop=(d == ND - 1))
                        # Stage both PSUM tiles via Act, then combine on the
                        # Pool engine: DVE stays free so its in-order stream
                        # reaches the first mq block during this pass.
                        sA = vstage.tile([P, WVCB], FP, tag="sA")
                        sB = vstage.tile([P, WVCB], FP, tag="sB")
                        nc.scalar.activation(sA, psA, AF.Copy)
                        nc.scalar.activation(sB, psB, AF.Copy)
                        for h in range(HPC):
                            c0 = h * DH
                            # vr_h = A[vr] - B[vi];  vi_h = A[vi] + B[vr]
                            nc.gpsimd.tensor_sub(vaug[h][:, tt, 0:DH],
                                                 sA[:, c0:c0 + DH],
                                                 sB[:, VC + c0:VC + c0 + DH])
                            nc.gpsimd.tensor_add(vaug[h][:, tt, DH:2 * DH],
                                                 sA[:, VC + c0:VC + c0 + DH],
                                                 sB[:, c0:c0 + DH])
                        # cb8: (A[cbr] - B[cbi])/8 , (A[cbi] + B[cbr])/8
                        # (tiny subs on DVE — on Pool they'd sit behind the
                        # long kTs adds and head-of-line-block Act's cb8
                        # copies, stalling the whole v/cb pass)
                        tr = cbtmp.tile([P, HPC], FP, tag="tr")
                        ti = cbtmp.tile([P, HPC], FP, tag="ti")
                        nc.vector.tensor_sub(tr, sA[:, 2 * VC:2 * VC + HPC],
                                             sB[:, 2 * VC + HPC:2 * VC + 2 * HPC])
                        nc.vector.tensor_add(ti, sA[:, 2 * VC + HPC:2 * VC + 2 * HPC],
                                             sB[:, 2 * VC:2 * VC + HPC])
                        # (tensor_scalar is not a legal Pool-engine opcode;
                        # these tiny muls go to Act as scaled copies)
                        cbc = tt * 2 * HPC
                        nc.scalar.activation(cb8[:, cbc:cbc + HPC], tr,
                                             AF.Copy, scale=0.125)
                        nc.scalar.activation(cb8[:, cbc + HPC:cbc + 2 * HPC], ti,
                                             AF.Copy, scale=0.125)
                ctx_vw.__exit__(None, None, None)

            # ---- phase S: per-head scores -> softmax -> context ---------
            kTs = hs  # computed in place at the end of phase P
            with (
                tc.tile_pool(name="mqp", bufs=2) as mqp,
                tc.tile_pool(name="ep", bufs=2) as ep,
                tc.tile_pool(name="etmp", bufs=2) as etmp,
                tc.tile_pool(name="psc", bufs=2, space="PSUM") as psc,
                tc.tile_pool(name="pctx", bufs=1, space="PSUM") as pctx,
                tc.tile_pool(name="ctxs", bufs=4) as ctxs,
            ):
                for h in range(HPC):
                    for ss in range(NSS):
                        ssl = slice(ss * SWS, (ss + 1) * SWS)
                        mqr = mqp.tile([P, NDK, SWS], FR, tag="mqr")
                        mqi = mqp.tile([P, NDK, SWS], FR, tag="mqi")
                        mqs = mqp.tile([P, NDK, SWS], FR, tag="mqs")
                        # grouped by kind: mqr completes first so the t1
                        # matmul group can start while mqi/mqs still build
                        for a in range(NDK):
                            mbase = (h * NDK + a) * 3
                            mr = mixv[:, mbase:mbase + 1]
                            min_ = mixv[:, mbase + 2:mbase + 3]
                            # mqr = qTr*mr - qTi*mi
                            nc.vector.tensor_scalar_mul(mqr[:, a], qTr[:, a, ssl], mr)
                            nc.vector.scalar_tensor_tensor(
                                mqr[:, a], qTi[:, a, ssl], min_, mqr[:, a],
                                op0=OP.mult, op1=OP.add)
                        for a in range(NDK):
                            mbase = (h * NDK + a) * 3
                            mr = mixv[:, mbase:mbase + 1]
                            mi = mixv[:, mbase + 1:mbase + 2]
                            # mqi = qTr*mi + qTi*mr
                            nc.vector.tensor_scalar_mul(mqi[:, a], qTr[:, a, ssl], mi)
                            nc.vector.scalar_tensor_tensor(
                                mqi[:, a], qTi[:, a, ssl], mr, mqi[:, a],
                                op0=OP.mult, op1=OP.add)
                        for a in range(NDK):
                            # Pool engine: keeps DVE (the hotter engine in
                            # phase S) off the mqs adds
                            nc.gpsimd.tensor_add(mqs[:, a], mqr[:, a], mqi[:, a])

                        Er = ep.tile([P, NT, SWS], HF, tag="Er")
                        Ei = ep.tile([P, NT, SWS], HF, tag="Ei")
                        for tt in range(NT):
                            tsl = slice(tt * P, (tt + 1) * P)
                            t1 = psc.tile([P, SWS], FP, tag="t1")
                            t2 = psc.tile([P, SWS], FP, tag="t2")
                            t3 = psc.tile([P, SWS], FP, tag="t3")
                            # grouped t1 -> t2 -> t3 so e1/e4 overlap t2/t3
                            for a in range(NDK):
                                nc.tensor.matmul(t1, kTr[:, a, tsl], mqr[:, a],
                                                 start=a == 0, stop=a == NDK - 1)
                            for a in range(NDK):
                                nc.tensor.matmul(t2, kTi[:, a, tsl], mqi[:, a],
                                                 start=a == 0, stop=a == NDK - 1)
                            for a in range(NDK):
                                nc.tensor.matmul(t3, kTs[:, a, tsl], mqs[:, a],
                                                 start=a == 0, stop=a == NDK - 1)
                            # Er = exp((t1-t2+cbr)/8) = e1*e2
                            # Ei = exp((t3-t1-t2+cbi)/8) = e3*e2*e4
                            cbc = tt * 2 * HPC
                            e1 = etmp.tile([P, SWS], HF, tag="e1")
                            e2 = etmp.tile([P, SWS], HF, tag="e2")
                            e3 = etmp.tile([P, SWS], HF, tag="e3")
                            e4 = etmp.tile([P, SWS], HF, tag="e4")
                            nc.scalar.activation(
                                e1, t1, AF.Exp,
                                bias=cb8[:, cbc + h:cbc + h + 1], scale=0.125)
                            nc.scalar.activation(e4, t1, AF.Exp, scale=-0.125)
                            nc.scalar.activation(e2, t2, AF.Exp, scale=-0.125)
                            nc.scalar.activation(
                                e3, t3, AF.Exp,
                                bias=cb8[:, cbc + HPC + h:cbc + HPC + h + 1],
                                scale=0.125)
                            nc.vector.tensor_mul(Er[:, tt], e1, e2)
                            m1 = etmp.tile([P, SWS], HF, tag="m1")
                            nc.vector.tensor_mul(m1, e3, e4)
                            nc.vector.tensor_mul(Ei[:, tt], m1, e2)

                        # context: for each 128-row block of queries
                        for sj in range(SWS // P):
                            st_idx = ss * (SWS // P) + sj
                            qsl = slice(sj * P, (sj + 1) * P)
                            pcA = pctx.tile([P, 2 * DH + 1], FP, tag="pcA")
                            pcB = pctx.tile([P, 2 * DH + 1], FP, tag="pcB")
                            for tt in range(NT):
                                st, sp = tt == 0, tt == NT - 1
                                nc.tensor.matmul(pcA, Er[:, tt, qsl], vaug[h][:, tt],
                                                 start=st, stop=sp)
                                nc.tensor.matmul(pcB, Ei[:, tt, qsl], vaug[h][:, tt],
                                                 start=st, stop=sp)
                            # Act stages the tiny context tiles out of PSUM
                            # (freeing the banks for the next sj) and takes
                            # the reciprocals; the Pool engine does the
                            # complex combine. DVE stays out of the ctx
                            # readout entirely.
                            sA2 = ctxs.tile([P, 2 * DH + 1], FP, tag="sA2")
                            sB2 = ctxs.tile([P, 2 * DH + 1], FP, tag="sB2")
                            nc.scalar.activation(sA2, pcA, AF.Copy)
                            nc.scalar.activation(sB2, pcB, AF.Copy)
                            rr = ctxs.tile([P, 1], FP, tag="rr")
                            ri = ctxs.tile([P, 1], FP, tag="ri")
                            nc.vector.reciprocal(rr, sA2[:, 2 * DH:2 * DH + 1])
                            nc.vector.reciprocal(ri, sB2[:, 2 * DH:2 * DH + 1])
                            # cr = A/sumr - Bvi/sumi ; ci = Avi/sumr + Bvr/sumi
                            tb = ctxs.tile([P, DH], FP, tag="tb")
                            td = ctxs.tile([P, DH], FP, tag="td")
                            cr = ctxs.tile([P, DH], FP, tag="cr")
                            ci = ctxs.tile([P, DH], FP, tag="ci")
                            nc.vector.tensor_scalar_mul(tb, sB2[:, DH:2 * DH], ri)
                            nc.vector.scalar_tensor_tensor(
                                cr, sA2[:, 0:DH], rr, tb, op0=OP.mult,
                                op1=OP.subtract)
                            nc.vector.tensor_scalar_mul(td, sB2[:, 0:DH], ri)
                            nc.vector.scalar_tensor_tensor(
                                ci, sA2[:, DH:2 * DH], rr, td, op0=OP.mult,
                                op1=OP.add)
                            nc.sync.dma_start(out_d[0, h, st_idx], cr)
                            nc.sync.dma_start(out_d[1, h, st_idx], ci)

            ctx_hsp.__exit__(None, None, None)

    if split_waits:
        _split_multi_waits(nc)
    return nc


def _prep_core_inputs(inputs, core):
    b = core // (N_CORES // B)
    hb = core % (N_CORES // B)
    heads = list(range(hb * HPC, (hb + 1) * HPC))
    cols = slice(hb * VC, (hb + 1) * VC)

    f32 = lambda x: np.ascontiguousarray(np.asarray(x, dtype=np.float32))
    hr = f32(inputs["hidden_r"][b]).T    # [D, S]
    hi = f32(inputs["hidden_i"][b]).T

    wv = np.concatenate(
        [f32(inputs["Wv_r"])[:, cols], f32(inputs["Wv_i"])[:, cols],
         f32(inputs["Wcb_r"])[:, heads], f32(inputs["Wcb_i"])[:, heads]], axis=1)
    bv = np.concatenate(
        [np.ones(P, np.float32),
         f32(inputs["bv_r"])[cols], f32(inputs["bv_i"])[cols],
         np.zeros(2 * HPC, np.float32)])

    mr = f32(inputs["mix_r"])[heads]     # [HPC, DK]
    mi = f32(inputs["mix_i"])[heads]
    mixv = np.stack([mr, mi, -mi], axis=-1)  # [HPC, DK, 3]

    c = np.ascontiguousarray
    return {
        "hTr": c(hr.reshape(ND, P, S)),
        "hTi": c(hi.reshape(ND, P, S)),
        "wqr": c(f32(inputs["Wq_r"]).reshape(ND, P, DK)),
        "wqi": c(f32(inputs["Wq_i"]).reshape(ND, P, DK)),
        "wkr": c(f32(inputs["Wk_r"]).reshape(ND, P, DK)),
        "wki": c(f32(inputs["Wk_i"]).reshape(ND, P, DK)),
        "wvcb": c(wv.reshape(ND, P, WVCB)),
        "bvec": c(bv.reshape(1, P + WVCB)),
        "mixv": c(mixv.reshape(HPC, NDK, P, 3).transpose(0, 1, 3, 2)),
    }


def kernel(**inputs):
    global _compiled, LAST_RESULTS
    if _compiled is None:
        _compiled = _build()
    nc = _compiled

    in_maps = [_prep_core_inputs(inputs, c) for c in range(N_CORES)]
    res = run_bass_kernel_spmd(nc, in_maps, core_ids=list(range(N_CORES)),
                               trace=TRACE)
    LAST_RESULTS = res

    out = np.zeros((2, B, S, DV), np.float32)
    for core in range(N_CORES):
        b = core // (N_CORES // B)
        hb = core % (N_CORES // B)
        oc = res.results[core]["out"]  # [2, HPC, NT, P, DH]
        for j in range(HPC):
            h = hb * HPC + j
            out[:, b, :, h * DH:(h + 1) * DH] = oc[:, j].reshape(2, S, DH)
    return out


# revision 55
# speedup vs baseline: 1.0001x; 1.0001x over previous
"""CollaborativeAttention (complex-valued, per-head mixed queries) on 8 trn2 cores.

Sharding: B*H = 24 (batch, head) units -> 3 heads per core.
  core c: batch b = c // 4, head block hb = c % 4 -> heads [3*hb, 3*hb+2].
Each core computes q/k projections for its batch (replicated within the
4-core batch group), v/cb projections for its head block only, then
scores+softmax+context for its 3 heads.

Karatsuba everywhere: a complex matmul (A_r+iA_i)(B_r+iB_i) is computed as
three real accumulation streams t1=A_r B_r, t2=A_i B_i, t3=(A_r+A_i)(B_r+B_i)
with real = t1-t2, imag = t3-t1-t2 — 25% less PE time than the 4-stream form.

 - Projections: hidden is fully resident in SBUF; hs=hr+hi is built by the
   Pool engine right behind the DMA stream, ws=wr+wi per weight tile by DVE,
   so every HBM byte is fetched exactly once on the (single-slot) DMA path,
   ordered so the matmul chains chase the stream. Combines: s2=copy(t2) on
   Act, then real = t1-s2, x = t3-s2, imag = x-t1 on DVE (one PSUM operand
   per op). kTs = kr+ki (the score Karatsuba stationary) overwrites the hs
   tile on Pool during the v/cb window.
 - Scores run transposed, sT[t, s] (key pos on partitions), in fp32r (full
   PE rate at >=256 moving rows). The exp is FACTORED instead of combined:
     Er = exp((t1-t2+cbr)/8) = exp((t1+cbr)/8) * exp(-t2/8) = e1*e2
     Ei = exp((t3-t1-t2+cbi)/8) = e3*e2*e4,  e4 = exp(-t1/8)
   so the Act engine reads each PSUM bank directly (4 exps, content bias
   fused as a per-partition ACT bias) and DVE does 3 cheap fp16 multiplies;
   no PSUM->SBUF staging pass is needed.
 - mixed queries: mqr/mqi via DVE tensor_scalar ops with per-partition
   mixing scalars; mqs = mqr+mqi on Pool. Double-buffered so the next
   (head, s-slice) block's DVE/Pool work overlaps this block's PE.
 - context matmul (moving dim 129 < 256) runs in fp16: probs and the
   per-head value matrix [vr | vi | 1] (ones column = softmax denominator);
   normalization happens on the tiny [128, 64] context tiles after an Act
   stage-out of PSUM (Pool has no PSUM port; tensor_scalar is not a legal
   Pool opcode — engine placement is ISA-constrained).
This walrus build encodes at most one sync-wait per instruction, so a
post-pass (_split_multi_waits) peels extra waits onto NoOps.
"""

import sys

for _p in ("/opt/trn_rl_repo", "/root/.axon_site", "/root/.axon_site/_ro/trn_rl_repo",
           "/root/.axon_site/_ro/pypackages"):
    if _p not in sys.path:
        sys.path.append(_p)

import numpy as np

import concourse.bass as bass
import concourse.mybir as mybir
import concourse.tile as tile
from concourse.bass_utils import run_bass_kernel_spmd

B, S, D, H = 2, 1024, 768, 12
DK = DV = 768
DH = DV // H          # 64 per-head value dim
HPC = 3               # heads per core
N_CORES = 8
P = 128
ND = D // P           # 6 d-tiles (contraction)
NDK = DK // P         # 6 output n-tiles for q/k
NT = S // P           # 8 token tiles
SWP = 512             # s-half width in projection PSUM tiles
SWS = 256             # s-slice width for scores
NSS = S // SWS        # 4 score s-slices
VC = HPC * DH         # 192 value cols per core
WVCB = 2 * VC + 2 * HPC  # 390: [Wv_r | Wv_i | Wcb_r | Wcb_i] cols

FP = mybir.dt.float32
FR = mybir.dt.float32r
HF = mybir.dt.float16
AF = mybir.ActivationFunctionType
OP = mybir.AluOpType

TRACE = False
LAST_RESULTS = None

_compiled = None


def _split_multi_waits(nc):
    """The walrus build here encodes at most ONE sync-wait per instruction
    ("Too many sync wait commands" in setupSyncWait otherwise). Tile freely
    emits several. Split the extras onto single-wait NoOps that precede the
    instruction in the same engine stream."""
    for fn in nc.m.functions:
        for bb in fn.blocks:
            out = []
            for ins in bb.instructions:
                si = ins.sync_info
                if si is not None and len(si.on_wait) > 1:
                    waits = list(si.on_wait)
                    for j, w in enumerate(waits[:-1]):
                        nop = mybir.InstNoOp(name=f"{ins.name}-ws{j}",
                                             ins=[], outs=[])
                        nop.engine = ins.engine
                        nop.sync_info = mybir.SyncInfo(on_wait=[w], on_update=[])
                        out.append(nop)
                    ins.sync_info = mybir.SyncInfo(on_wait=[waits[-1]],
                                                   on_update=list(si.on_update))
                out.append(ins)
            bb.instructions = out


def _build(split_waits=True):
    """Build the SPMD Bass program (identical on all 8 cores)."""
    nc = bass.Bass(trn_type="TRN2")

    hTr_d = nc.dram_tensor("hTr", [ND, P, S], FR, kind="ExternalInput")
    hTi_d = nc.dram_tensor("hTi", [ND, P, S], FR, kind="ExternalInput")
    wq_d = {c: nc.dram_tensor(f"wq{c}", [ND, P, DK], FR, kind="ExternalInput")
            for c in "ri"}
    wk_d = {c: nc.dram_tensor(f"wk{c}", [ND, P, DK], FR, kind="ExternalInput")
            for c in "ri"}
    wvcb_d = nc.dram_tensor("wvcb", [ND, P, WVCB], FR, kind="ExternalInput")
    bvec_d = nc.dram_tensor("bvec", [1, P + WVCB], FR, kind="ExternalInput")
    mixv_d = nc.dram_tensor("mixv", [HPC, NDK, 3, P], FP, kind="ExternalInput")
    out_d = nc.dram_tensor("out", [2, HPC, NT, P, DH], FP, kind="ExternalOutput")

    with tile.TileContext(nc) as tc:
        with (
            tc.tile_pool(name="persist", bufs=1) as persist,
            tc.tile_pool(name="vsmall", bufs=1) as vsmall,
        ):
            # ---- persistent tensors -------------------------------------
            # (kTs reuses the hs tile below — computed in place at the end
            # of phase P so no extra 24KB/partition is needed)
            qTr = persist.tile([P, NDK, S], FP)
            qTi = persist.tile([P, NDK, S], FP)
            kTr = persist.tile([P, NDK, S], FR)
            kTi = persist.tile([P, NDK, S], FR)

            # [mr | mi | -mi] per (h, a), col = (h*NDK + a)*3 + comp
            # On the Act queue: its SEQ only reaches this issue after real
            # work, keeping the single-slot DMA engine free at t=0 for the
            # weight/hidden loads the first matmuls block on.
            mixv = vsmall.tile([P, HPC * NDK * 3], FP)
            nc.scalar.dma_start(
                mixv, mixv_d[:].rearrange("h a c p -> p (h a c)"))
            # per-head context rhs: [vr_h | vi_h | 1]
            vaug = [vsmall.tile([P, NT, 2 * DH + 1], HF, tag=f"vaug{h}",
                                name=f"vaug{h}")
                    for h in range(HPC)]
            for h in range(HPC):
                nc.vector.memset(vaug[h][:, :, 2 * DH], 1.0)
            # (cbr/8 | cbi/8) per head, flattened: col = tt*2*HPC + (0|HPC) + h
            cb8 = vsmall.tile([P, NT * 2 * HPC], FP)

            # ---- phase P: projections -----------------------------------
            # hs holds hr+hi for the projection Karatsuba t3 streams; once
            # q/k are done it is overwritten with kTs = kr+ki (the score
            # Karatsuba stationary), so its 24KB/partition is reused and the
            # adds hide under the v/cb matmul window.
            ctx_hsp = tc.tile_pool(name="hsp", bufs=1)
            hsp = ctx_hsp.__enter__()
            hs = hsp.tile([P, ND, S], FR, tag="hs")
            with tc.tile_pool(name="hload", bufs=1) as hload:
                hr = hload.tile([P, ND, S], FR, tag="hr")
                hi = hload.tile([P, ND, S], FR, tag="hi")

                def _emit_hidden_half(j):
                    # Half-plane hidden DMAs on the same HWDGE queue as the
                    # weights, emitted between weight loads so the single
                    # DMA slot serves the matmul chains in consumption
                    # order. hs = hr+hi is built by the otherwise-idle Pool
                    # engine right behind the DMA stream instead of being
                    # shipped from host.
                    ssl = slice(j * SWP, (j + 1) * SWP)
                    for d in range(ND):
                        nc.sync.dma_start(hi[:, d, ssl], hTi_d[d, :, ssl])
                        nc.sync.dma_start(hr[:, d, ssl], hTr_d[d, :, ssl])
                    for d in range(ND):
                        nc.gpsimd.tensor_add(hs[:, d, ssl], hr[:, d, ssl],
                                             hi[:, d, ssl])

                # q/k projections: out[n, s] = sum_d W[d, n] * hT[d, s]
                ctx_vw = tc.tile_pool(name="vwides", bufs=1)
                vwides = ctx_vw.__enter__()
                with (
                    tc.tile_pool(name="wload", bufs=2) as wload,
                    tc.tile_pool(name="wsload", bufs=1) as wsload,
                    tc.tile_pool(name="pp23", bufs=1, space="PSUM") as pp23,
                    tc.tile_pool(name="pp1", bufs=2, space="PSUM") as pp1,
                    tc.tile_pool(name="stage", bufs=2) as stage,
                    tc.tile_pool(name="stagex", bufs=1) as stagex,
                ):
                    for (w_d, dst_r, dst_i, gname) in (
                        (wk_d, kTr, kTi, "k"),
                        (wq_d, qTr, qTi, "q"),
                    ):
                        for nt in range(NDK):
                            nsl = slice(nt * P, (nt + 1) * P)
                            t1 = [pp1.tile([P, SWP], FP, tag=f"t1s{j}",
                                           name=f"{gname}t1s{j}n{nt}")
                                  for j in range(2)]
                            t2 = [pp23.tile([P, SWP], FP, tag=f"t2s{j}",
                                            name=f"{gname}t2s{j}n{nt}")
                                  for j in range(2)]
                            t3 = [pp23.tile([P, SWP], FP, tag=f"t3s{j}",
                                            name=f"{gname}t3s{j}n{nt}")
                                  for j in range(2)]
                            wr = wload.tile([P, ND, P], FR, tag="wr")
                            wi = wload.tile([P, ND, P], FR, tag="wi")
                            ws = wsload.tile([P, ND, P], FR, tag="ws")
                            # wi first: the t2 chains consume it first
                            nc.sync.dma_start(
                                wi, w_d["i"][:, :, nsl].rearrange("a p c -> p a c"))
                            nc.sync.dma_start(
                                wr, w_d["r"][:, :, nsl].rearrange("a p c -> p a c"))
                            nc.vector.tensor_add(ws, wr, wi)
                            if gname == "k" and nt == 0:
                                # all hidden DMAs must precede the first
                                # chains in program order (deps are tracked
                                # in program order); nt0's weights went
                                # first on the queue so its chains start
                                # while hidden streams in behind them
                                _emit_hidden_half(0)
                                _emit_hidden_half(1)
                            # t2 chains first so their stops land earliest
                            # and the Act stage copy overlaps the t1/t3 tails
                            for src, tt_, htile in ((wi, t2, hi), (wr, t1, hr),
                                                    (ws, t3, hs)):
                                for j in range(2):
                                    ssl = slice(j * SWP, (j + 1) * SWP)
                                    for d in range(ND):
                                        nc.tensor.matmul(
                                            tt_[j], src[:, d], htile[:, d, ssl],
                                            start=d == 0, stop=d == ND - 1)
                            for j in range(2):
                                ssl = slice(j * SWP, (j + 1) * SWP)
                                s2 = stage.tile([P, SWP], FP, tag="s2")
                                nc.scalar.activation(s2, t2[j], AF.Copy)
                                # real = t1 - t2 ; imag = (t3 - t2) - t1
                                nc.vector.tensor_sub(dst_r[:, nt, ssl], t1[j], s2)
                                x = stagex.tile([P, SWP], FP, tag="x")
                                nc.vector.tensor_sub(x, t3[j], s2)
                                nc.vector.tensor_sub(dst_i[:, nt, ssl], x, t1[j])

                        if gname == "k":
                            # issue the v/cb-pass loads now, on the Act queue
                            # (reached after k's stage copies): clear of the
                            # startup DMA rush, landing long before the v/cb
                            # pass consumes them
                            bvec_sb = vwides.tile([1, P + WVCB], FR)
                            nc.scalar.dma_start(bvec_sb, bvec_d[:])
                            wvcb_sb = vwides.tile([P, ND, WVCB], FR)
                            nc.scalar.dma_start(
                                wvcb_sb, wvcb_d[:].rearrange("a p c -> p a c"))

                    # kTs = kr + ki, overwriting hs in place (program order
                    # is past q's t3 reads here). On the Pool engine so DVE
                    # reaches the first mq block during the v/cb pass.
                    for nt in range(NDK):
                        nc.gpsimd.tensor_add(hs[:, nt], kTr[:, nt], kTi[:, nt])



                # v / cb projections: [tok, c] = sum_d hT[d, tok] * Wbig[d, c]
                with (
                    tc.tile_pool(name="pv", bufs=2, space="PSUM") as pv,
                    tc.tile_pool(name="cbtmp", bufs=2) as cbtmp,
                    tc.tile_pool(name="vstage", bufs=4) as vstage,
                ):
                    for tt in range(NT):
                        tsl = slice(tt * P, (tt + 1) * P)
                        psA = pv.tile([P, WVCB], FP, tag="psA")
                        psB = pv.tile([P, WVCB], FP, tag="psB")
                        # bias row (bv | 0) into psA first (fewest deps first)
                        nc.tensor.matmul(psA, bvec_sb[:, :P], bvec_sb[:, P:],
                                         start=True, stop=False)
                        for d in range(ND):
                            nc.tensor.matmul(psA, hr[:, d, tsl], wvcb_sb[:, d],
                                             start=False, stop=(d == ND - 1))
                            nc.tensor.matmul(psB, hi[:, d, tsl], wvcb_sb[:, d],
                                             start=(d == 0), stop=(d == ND - 1))
                        # Stage both PSUM tiles via Act, then combine on the
                        # Pool engine: DVE stays free so its in-order stream
                        # reaches the first mq block during this pass.
                        sA = vstage.tile([P, WVCB], FP, tag="sA")
                        sB = vstage.tile([P, WVCB], FP, tag="sB")
                        nc.scalar.activation(sA, psA, AF.Copy)
                        nc.scalar.activation(sB, psB, AF.Copy)
                        for h in range(HPC):
                            c0 = h * DH
                            # vr_h = A[vr] - B[vi];  vi_h = A[vi] + B[vr]
                            nc.gpsimd.tensor_sub(vaug[h][:, tt, 0:DH],
                                                 sA[:, c0:c0 + DH],
                                                 sB[:, VC + c0:VC + c0 + DH])
                            nc.gpsimd.tensor_add(vaug[h][:, tt, DH:2 * DH],
                                                 sA[:, VC + c0:VC + c0 + DH],
                                                 sB[:, c0:c0 + DH])
                        # cb8: (A[cbr] - B[cbi])/8 , (A[cbi] + B[cbr])/8
                        # (tiny subs on DVE — on Pool they'd sit behind the
                        # long kTs adds and head-of-line-block Act's cb8
                        # copies, stalling the whole v/cb pass)
                        tr = cbtmp.tile([P, HPC], FP, tag="tr")
                        ti = cbtmp.tile([P, HPC], FP, tag="ti")
                        nc.vector.tensor_sub(tr, sA[:, 2 * VC:2 * VC + HPC],
                                             sB[:, 2 * VC + HPC:2 * VC + 2 * HPC])
                        nc.vector.tensor_add(ti, sA[:, 2 * VC + HPC:2 * VC + 2 * HPC],
                                             sB[:, 2 * VC:2 * VC + HPC])
                        # (tensor_scalar is not a legal Pool-engine opcode;
                        # these tiny muls go to Act as scaled copies)
                        cbc = tt * 2 * HPC
                        nc.scalar.activation(cb8[:, cbc:cbc + HPC], tr,
                                             AF.Copy, scale=0.125)
                        nc.scalar.activation(cb8[:, cbc + HPC:cbc + 2 * HPC], ti,
                                             AF.Copy, scale=0.125)
                ctx_vw.__exit__(None, None, None)

            # ---- phase S: per-head scores -> softmax -> context ---------
            kTs = hs  # computed in place at the end of phase P
            with (
                tc.tile_pool(name="mqp", bufs=2) as mqp,
                tc.tile_pool(name="ep", bufs=2) as ep,
                tc.tile_pool(name="etmp", bufs=2) as etmp,
                tc.tile_pool(name="psc", bufs=2, space="PSUM") as psc,
                tc.tile_pool(name="pctx", bufs=1, space="PSUM") as pctx,
                tc.tile_pool(name="ctxs", bufs=4) as ctxs,
            ):
                for h in range(HPC):
                    for ss in range(NSS):
                        ssl = slice(ss * SWS, (ss + 1) * SWS)
                        mqr = mqp.tile([P, NDK, SWS], FR, tag="mqr")
                        mqi = mqp.tile([P, NDK, SWS], FR, tag="mqi")
                        mqs = mqp.tile([P, NDK, SWS], FR, tag="mqs")
                        # grouped by kind: mqr completes first so the t1
                        # matmul group can start while mqi/mqs still build
                        for a in range(NDK):
                            mbase = (h * NDK + a) * 3
                            mr = mixv[:, mbase:mbase + 1]
                            min_ = mixv[:, mbase + 2:mbase + 3]
                            # mqr = qTr*mr - qTi*mi
                            nc.vector.tensor_scalar_mul(mqr[:, a], qTr[:, a, ssl], mr)
                            nc.vector.scalar_tensor_tensor(
                                mqr[:, a], qTi[:, a, ssl], min_, mqr[:, a],
                                op0=OP.mult, op1=OP.add)
                        for a in range(NDK):
                            mbase = (h * NDK + a) * 3
                            mr = mixv[:, mbase:mbase + 1]
                            mi = mixv[:, mbase + 1:mbase + 2]
                            # mqi = qTr*mi + qTi*mr
                            nc.vector.tensor_scalar_mul(mqi[:, a], qTr[:, a, ssl], mi)
                            nc.vector.scalar_tensor_tensor(
                                mqi[:, a], qTi[:, a, ssl], mr, mqi[:, a],
                                op0=OP.mult, op1=OP.add)
                        for a in range(NDK):
                            # Pool engine: keeps DVE (the hotter engine in
                            # phase S) off the mqs adds
                            nc.gpsimd.tensor_add(mqs[:, a], mqr[:, a], mqi[:, a])

                        Er = ep.tile([P, NT, SWS], HF, tag="Er")
                        Ei = ep.tile([P, NT, SWS], HF, tag="Ei")
                        for tt in range(NT):
                            tsl = slice(tt * P, (tt + 1) * P)
                            t1 = psc.tile([P, SWS], FP, tag="t1")
                            t2 = psc.tile([P, SWS], FP, tag="t2")
                            t3 = psc.tile([P, SWS], FP, tag="t3")
                            # grouped t1 -> t2 -> t3 so e1/e4 overlap t2/t3
                            for a in range(NDK):
                                nc.tensor.matmul(t1, kTr[:, a, tsl], mqr[:, a],
                                                 start=a == 0, stop=a == NDK - 1)
                            for a in range(NDK):
                                nc.tensor.matmul(t2, kTi[:, a, tsl], mqi[:, a],
                                                 start=a == 0, stop=a == NDK - 1)
                            for a in range(NDK):
                                nc.tensor.matmul(t3, kTs[:, a, tsl], mqs[:, a],
                                                 start=a == 0, stop=a == NDK - 1)
                            # Er = exp((t1-t2+cbr)/8) = e1*e2
                            # Ei = exp((t3-t1-t2+cbi)/8) = e3*e2*e4
                            cbc = tt * 2 * HPC
                            e1 = etmp.tile([P, SWS], HF, tag="e1")
                            e2 = etmp.tile([P, SWS], HF, tag="e2")
                            e3 = etmp.tile([P, SWS], HF, tag="e3")
                            e4 = etmp.tile([P, SWS], HF, tag="e4")
                            nc.scalar.activation(
                                e1, t1, AF.Exp,
                                bias=cb8[:, cbc + h:cbc + h + 1], scale=0.125)
                            nc.scalar.activation(e4, t1, AF.Exp, scale=-0.125)
                            nc.scalar.activation(e2, t2, AF.Exp, scale=-0.125)
                            nc.scalar.activation(
                                e3, t3, AF.Exp,
                                bias=cb8[:, cbc + HPC + h:cbc + HPC + h + 1],
                                scale=0.125)
                            nc.vector.tensor_mul(Er[:, tt], e1, e2)
                            m1 = etmp.tile([P, SWS], HF, tag="m1")
                            nc.vector.tensor_mul(m1, e3, e4)
                            nc.vector.tensor_mul(Ei[:, tt], m1, e2)

                        # context: for each 128-row block of queries
                        for sj in range(SWS // P):
                            st_idx = ss * (SWS // P) + sj
                            qsl = slice(sj * P, (sj + 1) * P)
                            pcA = pctx.tile([P, 2 * DH + 1], FP, tag="pcA")
                            pcB = pctx.tile([P, 2 * DH + 1], FP, tag="pcB")
                            for tt in range(NT):
                                st, sp = tt == 0, tt == NT - 1
                                nc.tensor.matmul(pcA, Er[:, tt, qsl], vaug[h][:, tt],
                                                 start=st, stop=sp)
                                nc.tensor.matmul(pcB, Ei[:, tt, qsl], vaug[h][:, tt],
                                                 start=st, stop=sp)
                            # Act stages the tiny context tiles out of PSUM
                            # (freeing the banks for the next sj) and takes
                            # the reciprocals; the Pool engine does the
                            # complex combine. DVE stays out of the ctx
                            # readout entirely.
                            sA2 = ctxs.tile([P, 2 * DH + 1], FP, tag="sA2")
                            sB2 = ctxs.tile([P, 2 * DH + 1], FP, tag="sB2")
                            nc.scalar.activation(sA2, pcA, AF.Copy)
                            nc.scalar.activation(sB2, pcB, AF.Copy)
                            rr = ctxs.tile([P, 1], FP, tag="rr")
                            ri = ctxs.tile([P, 1], FP, tag="ri")
                            nc.vector.reciprocal(rr, sA2[:, 2 * DH:2 * DH + 1])
                            nc.vector.reciprocal(ri, sB2[:, 2 * DH:2 * DH + 1])
                            # cr = A/sumr - Bvi/sumi ; ci = Avi/sumr + Bvr/sumi
                            tb = ctxs.tile([P, DH], FP, tag="tb")
                            td = ctxs.tile([P, DH], FP, tag="td")
                            cr = ctxs.tile([P, DH], FP, tag="cr")
                            ci = ctxs.tile([P, DH], FP, tag="ci")
                            nc.vector.tensor_scalar_mul(tb, sB2[:, DH:2 * DH], ri)
                            nc.vector.scalar_tensor_tensor(
                                cr, sA2[:, 0:DH], rr, tb, op0=OP.mult,
                                op1=OP.subtract)
                            nc.vector.tensor_scalar_mul(td, sB2[:, 0:DH], ri)
                            nc.vector.scalar_tensor_tensor(
                                ci, sA2[:, DH:2 * DH], rr, td, op0=OP.mult,
                                op1=OP.add)
                            nc.sync.dma_start(out_d[0, h, st_idx], cr)
                            nc.sync.dma_start(out_d[1, h, st_idx], ci)

            ctx_hsp.__exit__(None, None, None)

    if split_waits:
        _split_multi_waits(nc)
    return nc


def _prep_core_inputs(inputs, core):
    b = core // (N_CORES // B)
    hb = core % (N_CORES // B)
    heads = list(range(hb * HPC, (hb + 1) * HPC))
    cols = slice(hb * VC, (hb + 1) * VC)

    f32 = lambda x: np.ascontiguousarray(np.asarray(x, dtype=np.float32))
    hr = f32(inputs["hidden_r"][b]).T    # [D, S]
    hi = f32(inputs["hidden_i"][b]).T

    wv = np.concatenate(
        [f32(inputs["Wv_r"])[:, cols], f32(inputs["Wv_i"])[:, cols],
         f32(inputs["Wcb_r"])[:, heads], f32(inputs["Wcb_i"])[:, heads]], axis=1)
    bv = np.concatenate(
        [np.ones(P, np.float32),
         f32(inputs["bv_r"])[cols], f32(inputs["bv_i"])[cols],
         np.zeros(2 * HPC, np.float32)])

    mr = f32(inputs["mix_r"])[heads]     # [HPC, DK]
    mi = f32(inputs["mix_i"])[heads]
    mixv = np.stack([mr, mi, -mi], axis=-1)  # [HPC, DK, 3]

    c = np.ascontiguousarray
    return {
        "hTr": c(hr.reshape(ND, P, S)),
        "hTi": c(hi.reshape(ND, P, S)),
        "wqr": c(f32(inputs["Wq_r"]).reshape(ND, P, DK)),
        "wqi": c(f32(inputs["Wq_i"]).reshape(ND, P, DK)),
        "wkr": c(f32(inputs["Wk_r"]).reshape(ND, P, DK)),
        "wki": c(f32(inputs["Wk_i"]).reshape(ND, P, DK)),
        "wvcb": c(wv.reshape(ND, P, WVCB)),
        "bvec": c(bv.reshape(1, P + WVCB)),
        "mixv": c(mixv.reshape(HPC, NDK, P, 3).transpose(0, 1, 3, 2)),
    }


def kernel(**inputs):
    global _compiled, LAST_RESULTS
    if _compiled is None:
        _compiled = _build()
    nc = _compiled

    in_maps = [_prep_core_inputs(inputs, c) for c in range(N_CORES)]
    res = run_bass_kernel_spmd(nc, in_maps, core_ids=list(range(N_CORES)),
                               trace=TRACE)
    LAST_RESULTS = res

    out = np.zeros((2, B, S, DV), np.float32)
    for core in range(N_CORES):
        b = core // (N_CORES // B)
        hb = core % (N_CORES // B)
        oc = res.results[core]["out"]  # [2, HPC, NT, P, DH]
        for j in range(HPC):
            h = hb * HPC + j
            out[:, b, :, h * DH:(h + 1) * DH] = oc[:, j].reshape(2, S, DH)
    return out


# revision 57
# speedup vs baseline: 1.0022x; 1.0021x over previous
"""CollaborativeAttention (complex-valued, per-head mixed queries) on 8 trn2 cores.

Sharding: B*H = 24 (batch, head) units -> 3 heads per core.
  core c: batch b = c // 4, head block hb = c % 4 -> heads [3*hb, 3*hb+2].
Each core computes q/k projections for its batch (replicated within the
4-core batch group), v/cb projections for its head block only, then
scores+softmax+context for its 3 heads.

Karatsuba everywhere: a complex matmul (A_r+iA_i)(B_r+iB_i) is computed as
three real accumulation streams t1=A_r B_r, t2=A_i B_i, t3=(A_r+A_i)(B_r+B_i)
with real = t1-t2, imag = t3-t1-t2 — 25% less PE time than the 4-stream form.

 - Projections: hidden is fully resident in SBUF; hs=hr+hi is built by the
   Pool engine right behind the DMA stream, ws=wr+wi per weight tile by DVE,
   so every HBM byte is fetched exactly once on the (single-slot) DMA path,
   ordered so the matmul chains chase the stream. Combines: s2=copy(t2) on
   Act, then real = t1-s2, x = t3-s2, imag = x-t1 on DVE (one PSUM operand
   per op). kTs = kr+ki (the score Karatsuba stationary) overwrites the hs
   tile on Pool during the v/cb window.
 - Scores run transposed, sT[t, s] (key pos on partitions), in fp32r (full
   PE rate at >=256 moving rows). The exp is FACTORED instead of combined:
     Er = exp((t1-t2+cbr)/8) = exp((t1+cbr)/8) * exp(-t2/8) = e1*e2
     Ei = exp((t3-t1-t2+cbi)/8) = e3*e2*e4,  e4 = exp(-t1/8)
   so the Act engine reads each PSUM bank directly (4 exps, content bias
   fused as a per-partition ACT bias) and DVE does 3 cheap fp16 multiplies;
   no PSUM->SBUF staging pass is needed.
 - mixed queries: mqr/mqi via DVE tensor_scalar ops with per-partition
   mixing scalars; mqs = mqr+mqi on Pool. Double-buffered so the next
   (head, s-slice) block's DVE/Pool work overlaps this block's PE.
 - context matmul (moving dim 129 < 256) runs in fp16: probs and the
   per-head value matrix [vr | vi | 1] (ones column = softmax denominator);
   normalization happens on the tiny [128, 64] context tiles after an Act
   stage-out of PSUM (Pool has no PSUM port; tensor_scalar is not a legal
   Pool opcode — engine placement is ISA-constrained).
This walrus build encodes at most one sync-wait per instruction, so a
post-pass (_split_multi_waits) peels extra waits onto NoOps.
"""

import sys

for _p in ("/opt/trn_rl_repo", "/root/.axon_site", "/root/.axon_site/_ro/trn_rl_repo",
           "/root/.axon_site/_ro/pypackages"):
    if _p not in sys.path:
        sys.path.append(_p)

import numpy as np

import concourse.bass as bass
import concourse.mybir as mybir
import concourse.tile as tile
from concourse.bass_utils import run_bass_kernel_spmd

B, S, D, H = 2, 1024, 768, 12
DK = DV = 768
DH = DV // H          # 64 per-head value dim
HPC = 3               # heads per core
N_CORES = 8
P = 128
ND = D // P           # 6 d-tiles (contraction)
NDK = DK // P         # 6 output n-tiles for q/k
NT = S // P           # 8 token tiles
SWP = 512             # s-half width in projection PSUM tiles
SWS = 256             # s-slice width for scores
NSS = S // SWS        # 4 score s-slices
VC = HPC * DH         # 192 value cols per core
WVCB = 2 * VC + 2 * HPC  # 390: [Wv_r | Wv_i | Wcb_r | Wcb_i] cols

FP = mybir.dt.float32
FR = mybir.dt.float32r
HF = mybir.dt.float16
AF = mybir.ActivationFunctionType
OP = mybir.AluOpType

TRACE = False
LAST_RESULTS = None

_compiled = None


def _split_multi_waits(nc):
    """The walrus build here encodes at most ONE sync-wait per instruction
    ("Too many sync wait commands" in setupSyncWait otherwise). Tile freely
    emits several. Split the extras onto single-wait NoOps that precede the
    instruction in the same engine stream."""
    for fn in nc.m.functions:
        for bb in fn.blocks:
            out = []
            for ins in bb.instructions:
                si = ins.sync_info
                if si is not None and len(si.on_wait) > 1:
                    waits = list(si.on_wait)
                    for j, w in enumerate(waits[:-1]):
                        nop = mybir.InstNoOp(name=f"{ins.name}-ws{j}",
                                             ins=[], outs=[])
                        nop.engine = ins.engine
                        nop.sync_info = mybir.SyncInfo(on_wait=[w], on_update=[])
                        out.append(nop)
                    ins.sync_info = mybir.SyncInfo(on_wait=[waits[-1]],
                                                   on_update=list(si.on_update))
                out.append(ins)
            bb.instructions = out


def _build(split_waits=True):
    """Build the SPMD Bass program (identical on all 8 cores)."""
    nc = bass.Bass(trn_type="TRN2")

    hTr_d = nc.dram_tensor("hTr", [ND, P, S], FR, kind="ExternalInput")
    hTi_d = nc.dram_tensor("hTi", [ND, P, S], FR, kind="ExternalInput")
    wq_d = {c: nc.dram_tensor(f"wq{c}", [ND, P, DK], FR, kind="ExternalInput")
            for c in "ri"}
    wk_d = {c: nc.dram_tensor(f"wk{c}", [ND, P, DK], FR, kind="ExternalInput")
            for c in "ri"}
    wvcb_d = nc.dram_tensor("wvcb", [ND, P, WVCB], FR, kind="ExternalInput")
    bvec_d = nc.dram_tensor("bvec", [1, P + WVCB], FR, kind="ExternalInput")
    mixv_d = nc.dram_tensor("mixv", [HPC, NDK, 3, P], FP, kind="ExternalInput")
    out_d = nc.dram_tensor("out", [2, HPC, NT, P, DH], FP, kind="ExternalOutput")

    with tile.TileContext(nc) as tc:
        with (
            tc.tile_pool(name="persist", bufs=1) as persist,
            tc.tile_pool(name="vsmall", bufs=1) as vsmall,
        ):
            # ---- persistent tensors -------------------------------------
            # (kTs reuses the hs tile below — computed in place at the end
            # of phase P so no extra 24KB/partition is needed)
            qTr = persist.tile([P, NDK, S], FP)
            qTi = persist.tile([P, NDK, S], FP)
            kTr = persist.tile([P, NDK, S], FR)
            kTi = persist.tile([P, NDK, S], FR)

            # [mr | mi | -mi] per (h, a), col = (h*NDK + a)*3 + comp
            # On the Act queue: its SEQ only reaches this issue after real
            # work, keeping the single-slot DMA engine free at t=0 for the
            # weight/hidden loads the first matmuls block on.
            mixv = vsmall.tile([P, HPC * NDK * 3], FP)
            nc.scalar.dma_start(
                mixv, mixv_d[:].rearrange("h a c p -> p (h a c)"))
            # per-head context rhs: [vr_h | vi_h | 1]
            vaug = [vsmall.tile([P, NT, 2 * DH + 1], HF, tag=f"vaug{h}",
                                name=f"vaug{h}")
                    for h in range(HPC)]
            for h in range(HPC):
                nc.vector.memset(vaug[h][:, :, 2 * DH], 1.0)
            # (cbr/8 | cbi/8) per head, flattened: col = tt*2*HPC + (0|HPC) + h
            cb8 = vsmall.tile([P, NT * 2 * HPC], FP)

            # ---- phase P: projections -----------------------------------
            # hs holds hr+hi for the projection Karatsuba t3 streams; once
            # q/k are done it is overwritten with kTs = kr+ki (the score
            # Karatsuba stationary), so its 24KB/partition is reused and the
            # adds hide under the v/cb matmul window.
            ctx_hsp = tc.tile_pool(name="hsp", bufs=1)
            hsp = ctx_hsp.__enter__()
            hs = hsp.tile([P, ND, S], FR, tag="hs")
            with tc.tile_pool(name="hload", bufs=1) as hload:
                hr = hload.tile([P, ND, S], FR, tag="hr")
                hi = hload.tile([P, ND, S], FR, tag="hi")

                def _emit_hidden_half(j):
                    # Half-plane hidden DMAs on the same HWDGE queue as the
                    # weights, emitted between weight loads so the single
                    # DMA slot serves the matmul chains in consumption
                    # order. hs = hr+hi is built by the otherwise-idle Pool
                    # engine right behind the DMA stream instead of being
                    # shipped from host.
                    ssl = slice(j * SWP, (j + 1) * SWP)
                    for d in range(ND):
                        nc.sync.dma_start(hi[:, d, ssl], hTi_d[d, :, ssl])
                        nc.sync.dma_start(hr[:, d, ssl], hTr_d[d, :, ssl])
                    for d in range(ND):
                        nc.gpsimd.tensor_add(hs[:, d, ssl], hr[:, d, ssl],
                                             hi[:, d, ssl])

                # q/k projections: out[n, s] = sum_d W[d, n] * hT[d, s]
                ctx_vw = tc.tile_pool(name="vwides", bufs=1)
                vwides = ctx_vw.__enter__()
                with (
                    tc.tile_pool(name="wload", bufs=2) as wload,
                    tc.tile_pool(name="wsload", bufs=1) as wsload,
                    tc.tile_pool(name="pp23", bufs=1, space="PSUM") as pp23,
                    tc.tile_pool(name="pp1", bufs=2, space="PSUM") as pp1,
                    tc.tile_pool(name="stage", bufs=2) as stage,
                    tc.tile_pool(name="stagex", bufs=1) as stagex,
                ):
                    for (w_d, dst_r, dst_i, gname) in (
                        (wk_d, kTr, kTi, "k"),
                        (wq_d, qTr, qTi, "q"),
                    ):
                        def _load_w(nt, w_d=w_d):
                            # wi first: the t2 chains consume it first
                            nsl_ = slice(nt * P, (nt + 1) * P)
                            wi_ = wload.tile([P, ND, P], FR, tag="wi")
                            wr_ = wload.tile([P, ND, P], FR, tag="wr")
                            nc.sync.dma_start(
                                wi_,
                                w_d["i"][:, :, nsl_].rearrange("a p c -> p a c"))
                            nc.sync.dma_start(
                                wr_,
                                w_d["r"][:, :, nsl_].rearrange("a p c -> p a c"))
                            return wi_, wr_

                        # 2-deep weight prefetch: nt0 AND nt1 weights go on
                        # the (in-order, single-slot) DMA path before the
                        # 6.3MB of hidden, so nt1's chains don't stall
                        # behind the hidden stream.
                        pend = {0: _load_w(0), 1: _load_w(1)}
                        if gname == "k":
                            # all hidden DMAs must precede the first chains
                            # in program order (deps are tracked in program
                            # order)
                            _emit_hidden_half(0)
                            _emit_hidden_half(1)
                        for nt in range(NDK):
                            t1 = [pp1.tile([P, SWP], FP, tag=f"t1s{j}",
                                           name=f"{gname}t1s{j}n{nt}")
                                  for j in range(2)]
                            t2 = [pp23.tile([P, SWP], FP, tag=f"t2s{j}",
                                            name=f"{gname}t2s{j}n{nt}")
                                  for j in range(2)]
                            t3 = [pp23.tile([P, SWP], FP, tag=f"t3s{j}",
                                            name=f"{gname}t3s{j}n{nt}")
                                  for j in range(2)]
                            wi, wr = pend.pop(nt)
                            if nt + 2 < NDK:
                                pend[nt + 2] = _load_w(nt + 2)
                            ws = wsload.tile([P, ND, P], FR, tag="ws")
                            nc.vector.tensor_add(ws, wr, wi)
                            # t2 chains first so their stops land earliest
                            # and the Act stage copy overlaps the t1/t3 tails
                            for src, tt_, htile in ((wi, t2, hi), (wr, t1, hr),
                                                    (ws, t3, hs)):
                                for j in range(2):
                                    ssl = slice(j * SWP, (j + 1) * SWP)
                                    for d in range(ND):
                                        nc.tensor.matmul(
                                            tt_[j], src[:, d], htile[:, d, ssl],
                                            start=d == 0, stop=d == ND - 1)
                            for j in range(2):
                                ssl = slice(j * SWP, (j + 1) * SWP)
                                s2 = stage.tile([P, SWP], FP, tag="s2")
                                nc.scalar.activation(s2, t2[j], AF.Copy)
                                # real = t1 - t2 ; imag = (t3 - t2) - t1
                                nc.vector.tensor_sub(dst_r[:, nt, ssl], t1[j], s2)
                                x = stagex.tile([P, SWP], FP, tag="x")
                                nc.vector.tensor_sub(x, t3[j], s2)
                                nc.vector.tensor_sub(dst_i[:, nt, ssl], x, t1[j])

                        if gname == "k":
                            # issue the v/cb-pass loads now, on the Act queue
                            # (reached after k's stage copies): clear of the
                            # startup DMA rush, landing long before the v/cb
                            # pass consumes them
                            bvec_sb = vwides.tile([1, P + WVCB], FR)
                            nc.scalar.dma_start(bvec_sb, bvec_d[:])
                            wvcb_sb = vwides.tile([P, ND, WVCB], FR)
                            nc.scalar.dma_start(
                                wvcb_sb, wvcb_d[:].rearrange("a p c -> p a c"))

                    # kTs = kr + ki, overwriting hs in place (program order
                    # is past q's t3 reads here). On the Pool engine so DVE
                    # reaches the first mq block during the v/cb pass.
                    for nt in range(NDK):
                        nc.gpsimd.tensor_add(hs[:, nt], kTr[:, nt], kTi[:, nt])



                # v / cb projections: [tok, c] = sum_d hT[d, tok] * Wbig[d, c]
                with (
                    tc.tile_pool(name="pv", bufs=2, space="PSUM") as pv,
                    tc.tile_pool(name="cbtmp", bufs=2) as cbtmp,
                    tc.tile_pool(name="vstage", bufs=4) as vstage,
                ):
                    for tt in range(NT):
                        tsl = slice(tt * P, (tt + 1) * P)
                        psA = pv.tile([P, WVCB], FP, tag="psA")
                        psB = pv.tile([P, WVCB], FP, tag="psB")
                        # bias row (bv | 0) into psA first (fewest deps first)
                        nc.tensor.matmul(psA, bvec_sb[:, :P], bvec_sb[:, P:],
                                         start=True, stop=False)
                        for d in range(ND):
                            nc.tensor.matmul(psA, hr[:, d, tsl], wvcb_sb[:, d],
                                             start=False, stop=(d == ND - 1))
                            nc.tensor.matmul(psB, hi[:, d, tsl], wvcb_sb[:, d],
                                             start=(d == 0), stop=(d == ND - 1))
                        # Stage both PSUM tiles via Act, then combine on the
                        # Pool engine: DVE stays free so its in-order stream
                        # reaches the first mq block during this pass.
                        sA = vstage.tile([P, WVCB], FP, tag="sA")
                        sB = vstage.tile([P, WVCB], FP, tag="sB")
                        nc.scalar.activation(sA, psA, AF.Copy)
                        nc.scalar.activation(sB, psB, AF.Copy)
                        for h in range(HPC):
                            c0 = h * DH
                            # vr_h = A[vr] - B[vi];  vi_h = A[vi] + B[vr]
                            nc.gpsimd.tensor_sub(vaug[h][:, tt, 0:DH],
                                                 sA[:, c0:c0 + DH],
                                                 sB[:, VC + c0:VC + c0 + DH])
                            nc.gpsimd.tensor_add(vaug[h][:, tt, DH:2 * DH],
                                                 sA[:, VC + c0:VC + c0 + DH],
                                                 sB[:, c0:c0 + DH])
                        # cb8: (A[cbr] - B[cbi])/8 , (A[cbi] + B[cbr])/8
                        # (tiny subs on DVE — on Pool they'd sit behind the
                        # long kTs adds and head-of-line-block Act's cb8
                        # copies, stalling the whole v/cb pass)
                        tr = cbtmp.tile([P, HPC], FP, tag="tr")
                        ti = cbtmp.tile([P, HPC], FP, tag="ti")
                        nc.vector.tensor_sub(tr, sA[:, 2 * VC:2 * VC + HPC],
                                             sB[:, 2 * VC + HPC:2 * VC + 2 * HPC])
                        nc.vector.tensor_add(ti, sA[:, 2 * VC + HPC:2 * VC + 2 * HPC],
                                             sB[:, 2 * VC:2 * VC + HPC])
                        # (tensor_scalar is not a legal Pool-engine opcode;
                        # these tiny muls go to Act as scaled copies)
                        cbc = tt * 2 * HPC
                        nc.scalar.activation(cb8[:, cbc:cbc + HPC], tr,
                                             AF.Copy, scale=0.125)
                        nc.scalar.activation(cb8[:, cbc + HPC:cbc + 2 * HPC], ti,
                                             AF.Copy, scale=0.125)
                ctx_vw.__exit__(None, None, None)

            # ---- phase S: per-head scores -> softmax -> context ---------
            kTs = hs  # computed in place at the end of phase P
            with (
                tc.tile_pool(name="mqp", bufs=2) as mqp,
                tc.tile_pool(name="ep", bufs=2) as ep,
                tc.tile_pool(name="etmp", bufs=4) as etmp,
                tc.tile_pool(name="psc", bufs=2, space="PSUM") as psc,
                tc.tile_pool(name="pctx", bufs=1, space="PSUM") as pctx,
                tc.tile_pool(name="ctxs", bufs=4) as ctxs,
            ):
                for h in range(HPC):
                    for ss in range(NSS):
                        ssl = slice(ss * SWS, (ss + 1) * SWS)
                        mqr = mqp.tile([P, NDK, SWS], FR, tag="mqr")
                        mqi = mqp.tile([P, NDK, SWS], FR, tag="mqi")
                        mqs = mqp.tile([P, NDK, SWS], FR, tag="mqs")
                        # grouped by kind: mqr completes first so the t1
                        # matmul group can start while mqi/mqs still build
                        for a in range(NDK):
                            mbase = (h * NDK + a) * 3
                            mr = mixv[:, mbase:mbase + 1]
                            min_ = mixv[:, mbase + 2:mbase + 3]
                            # mqr = qTr*mr - qTi*mi
                            nc.vector.tensor_scalar_mul(mqr[:, a], qTr[:, a, ssl], mr)
                            nc.vector.scalar_tensor_tensor(
                                mqr[:, a], qTi[:, a, ssl], min_, mqr[:, a],
                                op0=OP.mult, op1=OP.add)
                        for a in range(NDK):
                            mbase = (h * NDK + a) * 3
                            mr = mixv[:, mbase:mbase + 1]
                            mi = mixv[:, mbase + 1:mbase + 2]
                            # mqi = qTr*mi + qTi*mr
                            nc.vector.tensor_scalar_mul(mqi[:, a], qTr[:, a, ssl], mi)
                            nc.vector.scalar_tensor_tensor(
                                mqi[:, a], qTi[:, a, ssl], mr, mqi[:, a],
                                op0=OP.mult, op1=OP.add)
                        for a in range(NDK):
                            # Pool engine: keeps DVE (the hotter engine in
                            # phase S) off the mqs adds
                            nc.gpsimd.tensor_add(mqs[:, a], mqr[:, a], mqi[:, a])

                        Er = ep.tile([P, NT, SWS], HF, tag="Er")
                        Ei = ep.tile([P, NT, SWS], HF, tag="Ei")
                        for tt in range(NT):
                            tsl = slice(tt * P, (tt + 1) * P)
                            t1 = psc.tile([P, SWS], FP, tag="t1")
                            t2 = psc.tile([P, SWS], FP, tag="t2")
                            t3 = psc.tile([P, SWS], FP, tag="t3")
                            # grouped t1 -> t2 -> t3 so e1/e4 overlap t2/t3
                            for a in range(NDK):
                                nc.tensor.matmul(t1, kTr[:, a, tsl], mqr[:, a],
                                                 start=a == 0, stop=a == NDK - 1)
                            for a in range(NDK):
                                nc.tensor.matmul(t2, kTi[:, a, tsl], mqi[:, a],
                                                 start=a == 0, stop=a == NDK - 1)
                            for a in range(NDK):
                                nc.tensor.matmul(t3, kTs[:, a, tsl], mqs[:, a],
                                                 start=a == 0, stop=a == NDK - 1)
                            # Er = exp((t1-t2+cbr)/8) = e1*e2
                            # Ei = exp((t3-t1-t2+cbi)/8) = e3*e2*e4
                            cbc = tt * 2 * HPC
                            e1 = etmp.tile([P, SWS], HF, tag="e1")
                            e2 = etmp.tile([P, SWS], HF, tag="e2")
                            e3 = etmp.tile([P, SWS], HF, tag="e3")
                            e4 = etmp.tile([P, SWS], HF, tag="e4")
                            nc.scalar.activation(
                                e1, t1, AF.Exp,
                                bias=cb8[:, cbc + h:cbc + h + 1], scale=0.125)
                            nc.scalar.activation(e4, t1, AF.Exp, scale=-0.125)
                            nc.scalar.activation(e2, t2, AF.Exp, scale=-0.125)
                            nc.scalar.activation(
                                e3, t3, AF.Exp,
                                bias=cb8[:, cbc + HPC + h:cbc + HPC + h + 1],
                                scale=0.125)
                            nc.vector.tensor_mul(Er[:, tt], e1, e2)
                            m1 = etmp.tile([P, SWS], HF, tag="m1")
                            nc.vector.tensor_mul(m1, e3, e4)
                            nc.vector.tensor_mul(Ei[:, tt], m1, e2)

                        # context: for each 128-row block of queries
                        for sj in range(SWS // P):
                            st_idx = ss * (SWS // P) + sj
                            qsl = slice(sj * P, (sj + 1) * P)
                            pcA = pctx.tile([P, 2 * DH + 1], FP, tag="pcA")
                            pcB = pctx.tile([P, 2 * DH + 1], FP, tag="pcB")
                            for tt in range(NT):
                                st, sp = tt == 0, tt == NT - 1
                                nc.tensor.matmul(pcA, Er[:, tt, qsl], vaug[h][:, tt],
                                                 start=st, stop=sp)
                                nc.tensor.matmul(pcB, Ei[:, tt, qsl], vaug[h][:, tt],
                                                 start=st, stop=sp)
                            # Act stages the tiny context tiles out of PSUM
                            # (freeing the banks for the next sj) and takes
                            # the reciprocals; the Pool engine does the
                            # complex combine. DVE stays out of the ctx
                            # readout entirely.
                            sA2 = ctxs.tile([P, 2 * DH + 1], FP, tag="sA2")
                            sB2 = ctxs.tile([P, 2 * DH + 1], FP, tag="sB2")
                            nc.scalar.activation(sA2, pcA, AF.Copy)
                            nc.scalar.activation(sB2, pcB, AF.Copy)
                            rr = ctxs.tile([P, 1], FP, tag="rr")
                            ri = ctxs.tile([P, 1], FP, tag="ri")
                            nc.vector.reciprocal(rr, sA2[:, 2 * DH:2 * DH + 1])
                            nc.vector.reciprocal(ri, sB2[:, 2 * DH:2 * DH + 1])
                            # cr = A/sumr - Bvi/sumi ; ci = Avi/sumr + Bvr/sumi
                            tb = ctxs.tile([P, DH], FP, tag="tb")
                            td = ctxs.tile([P, DH], FP, tag="td")
                            cr = ctxs.tile([P, DH], FP, tag="cr")
                            ci = ctxs.tile([P, DH], FP, tag="ci")
                            nc.vector.tensor_scalar_mul(tb, sB2[:, DH:2 * DH], ri)
                            nc.vector.scalar_tensor_tensor(
                                cr, sA2[:, 0:DH], rr, tb, op0=OP.mult,
                                op1=OP.subtract)
                            nc.vector.tensor_scalar_mul(td, sB2[:, 0:DH], ri)
                            nc.vector.scalar_tensor_tensor(
                                ci, sA2[:, DH:2 * DH], rr, td, op0=OP.mult,
                                op1=OP.add)
                            nc.sync.dma_start(out_d[0, h, st_idx], cr)
                            nc.sync.dma_start(out_d[1, h, st_idx], ci)

            ctx_hsp.__exit__(None, None, None)

    if split_waits:
        _split_multi_waits(nc)
    return nc


def _prep_core_inputs(inputs, core):
    b = core // (N_CORES // B)
    hb = core % (N_CORES // B)
    heads = list(range(hb * HPC, (hb + 1) * HPC))
    cols = slice(hb * VC, (hb + 1) * VC)

    f32 = lambda x: np.ascontiguousarray(np.asarray(x, dtype=np.float32))
    hr = f32(inputs["hidden_r"][b]).T    # [D, S]
    hi = f32(inputs["hidden_i"][b]).T

    wv = np.concatenate(
        [f32(inputs["Wv_r"])[:, cols], f32(inputs["Wv_i"])[:, cols],
         f32(inputs["Wcb_r"])[:, heads], f32(inputs["Wcb_i"])[:, heads]], axis=1)
    bv = np.concatenate(
        [np.ones(P, np.float32),
         f32(inputs["bv_r"])[cols], f32(inputs["bv_i"])[cols],
         np.zeros(2 * HPC, np.float32)])

    mr = f32(inputs["mix_r"])[heads]     # [HPC, DK]
    mi = f32(inputs["mix_i"])[heads]
    mixv = np.stack([mr, mi, -mi], axis=-1)  # [HPC, DK, 3]

    c = np.ascontiguousarray
    return {
        "hTr": c(hr.reshape(ND, P, S)),
        "hTi": c(hi.reshape(ND, P, S)),
        "wqr": c(f32(inputs["Wq_r"]).reshape(ND, P, DK)),
        "wqi": c(f32(inputs["Wq_i"]).reshape(ND, P, DK)),
        "wkr": c(f32(inputs["Wk_r"]).reshape(ND, P, DK)),
        "wki": c(f32(inputs["Wk_i"]).reshape(ND, P, DK)),
        "wvcb": c(wv.reshape(ND, P, WVCB)),
        "bvec": c(bv.reshape(1, P + WVCB)),
        "mixv": c(mixv.reshape(HPC, NDK, P, 3).transpose(0, 1, 3, 2)),
    }


def kernel(**inputs):
    global _compiled, LAST_RESULTS
    if _compiled is None:
        _compiled = _build()
    nc = _compiled

    in_maps = [_prep_core_inputs(inputs, c) for c in range(N_CORES)]
    res = run_bass_kernel_spmd(nc, in_maps, core_ids=list(range(N_CORES)),
                               trace=TRACE)
    LAST_RESULTS = res

    out = np.zeros((2, B, S, DV), np.float32)
    for core in range(N_CORES):
        b = core // (N_CORES // B)
        hb = core % (N_CORES // B)
        oc = res.results[core]["out"]  # [2, HPC, NT, P, DH]
        for j in range(HPC):
            h = hb * HPC + j
            out[:, b, :, h * DH:(h + 1) * DH] = oc[:, j].reshape(2, S, DH)
    return out


# revision 58
# speedup vs baseline: 1.0099x; 1.0077x over previous
"""CollaborativeAttention (complex-valued, per-head mixed queries) on 8 trn2 cores.

Sharding: B*H = 24 (batch, head) units -> 3 heads per core.
  core c: batch b = c // 4, head block hb = c % 4 -> heads [3*hb, 3*hb+2].
Each core computes q/k projections for its batch (replicated within the
4-core batch group), v/cb projections for its head block only, then
scores+softmax+context for its 3 heads.

Karatsuba everywhere: a complex matmul (A_r+iA_i)(B_r+iB_i) is computed as
three real accumulation streams t1=A_r B_r, t2=A_i B_i, t3=(A_r+A_i)(B_r+B_i)
with real = t1-t2, imag = t3-t1-t2 — 25% less PE time than the 4-stream form.

 - Projections: hidden is fully resident in SBUF; hs=hr+hi is built by the
   Pool engine right behind the DMA stream, ws=wr+wi per weight tile by DVE,
   so every HBM byte is fetched exactly once on the (single-slot) DMA path,
   ordered so the matmul chains chase the stream. Combines: s2=copy(t2) on
   Act, then real = t1-s2, x = t3-s2, imag = x-t1 on DVE (one PSUM operand
   per op). kTs = kr+ki (the score Karatsuba stationary) overwrites the hs
   tile on Pool during the v/cb window.
 - Scores run transposed, sT[t, s] (key pos on partitions), in fp32r (full
   PE rate at >=256 moving rows). The exp is FACTORED instead of combined:
     Er = exp((t1-t2+cbr)/8) = exp((t1+cbr)/8) * exp(-t2/8) = e1*e2
     Ei = exp((t3-t1-t2+cbi)/8) = e3*e2*e4,  e4 = exp(-t1/8)
   so the Act engine reads each PSUM bank directly (4 exps, content bias
   fused as a per-partition ACT bias) and DVE does 3 cheap fp16 multiplies;
   no PSUM->SBUF staging pass is needed.
 - mixed queries: mqr/mqi via DVE tensor_scalar ops with per-partition
   mixing scalars; mqs = mqr+mqi on Pool. Double-buffered so the next
   (head, s-slice) block's DVE/Pool work overlaps this block's PE.
 - context matmul (moving dim 129 < 256) runs in fp16: probs and the
   per-head value matrix [vr | vi | 1] (ones column = softmax denominator);
   normalization happens on the tiny [128, 64] context tiles after an Act
   stage-out of PSUM (Pool has no PSUM port; tensor_scalar is not a legal
   Pool opcode — engine placement is ISA-constrained).
This walrus build encodes at most one sync-wait per instruction, so a
post-pass (_split_multi_waits) peels extra waits onto NoOps.
"""

import sys

for _p in ("/opt/trn_rl_repo", "/root/.axon_site", "/root/.axon_site/_ro/trn_rl_repo",
           "/root/.axon_site/_ro/pypackages"):
    if _p not in sys.path:
        sys.path.append(_p)

import numpy as np

import concourse.bass as bass
import concourse.mybir as mybir
import concourse.tile as tile
from concourse.bass_utils import run_bass_kernel_spmd

B, S, D, H = 2, 1024, 768, 12
DK = DV = 768
DH = DV // H          # 64 per-head value dim
HPC = 3               # heads per core
N_CORES = 8
P = 128
ND = D // P           # 6 d-tiles (contraction)
NDK = DK // P         # 6 output n-tiles for q/k
NT = S // P           # 8 token tiles
SWP = 512             # s-half width in projection PSUM tiles
SWS = 256             # s-slice width for scores
NSS = S // SWS        # 4 score s-slices
VC = HPC * DH         # 192 value cols per core
WVCB = 2 * VC + 2 * HPC  # 390: [Wv_r | Wv_i | Wcb_r | Wcb_i] cols

FP = mybir.dt.float32
FR = mybir.dt.float32r
HF = mybir.dt.float16
AF = mybir.ActivationFunctionType
OP = mybir.AluOpType

TRACE = False
LAST_RESULTS = None

_compiled = None


def _split_multi_waits(nc):
    """The walrus build here encodes at most ONE sync-wait per instruction
    ("Too many sync wait commands" in setupSyncWait otherwise). Tile freely
    emits several. Split the extras onto single-wait NoOps that precede the
    instruction in the same engine stream."""
    for fn in nc.m.functions:
        for bb in fn.blocks:
            out = []
            for ins in bb.instructions:
                si = ins.sync_info
                if si is not None and len(si.on_wait) > 1:
                    waits = list(si.on_wait)
                    for j, w in enumerate(waits[:-1]):
                        nop = mybir.InstNoOp(name=f"{ins.name}-ws{j}",
                                             ins=[], outs=[])
                        nop.engine = ins.engine
                        nop.sync_info = mybir.SyncInfo(on_wait=[w], on_update=[])
                        out.append(nop)
                    ins.sync_info = mybir.SyncInfo(on_wait=[waits[-1]],
                                                   on_update=list(si.on_update))
                out.append(ins)
            bb.instructions = out


def _build(split_waits=True):
    """Build the SPMD Bass program (identical on all 8 cores)."""
    nc = bass.Bass(trn_type="TRN2")

    hTr_d = nc.dram_tensor("hTr", [ND, P, S], FR, kind="ExternalInput")
    hTi_d = nc.dram_tensor("hTi", [ND, P, S], FR, kind="ExternalInput")
    wq_d = {c: nc.dram_tensor(f"wq{c}", [ND, P, DK], FR, kind="ExternalInput")
            for c in "ri"}
    wk_d = {c: nc.dram_tensor(f"wk{c}", [ND, P, DK], FR, kind="ExternalInput")
            for c in "ri"}
    wvcb_d = nc.dram_tensor("wvcb", [ND, P, WVCB], FR, kind="ExternalInput")
    bvec_d = nc.dram_tensor("bvec", [1, P + WVCB], FR, kind="ExternalInput")
    mixv_d = nc.dram_tensor("mixv", [P, HPC * NDK * 3], FP, kind="ExternalInput")
    out_d = nc.dram_tensor("out", [2, HPC, NT, P, DH], FP, kind="ExternalOutput")

    with tile.TileContext(nc) as tc:
        with (
            tc.tile_pool(name="persist", bufs=1) as persist,
            tc.tile_pool(name="vsmall", bufs=1) as vsmall,
        ):
            # ---- persistent tensors -------------------------------------
            # (kTs reuses the hs tile below — computed in place at the end
            # of phase P so no extra 24KB/partition is needed)
            qTr = persist.tile([P, NDK, S], FP)
            qTi = persist.tile([P, NDK, S], FP)
            kTr = persist.tile([P, NDK, S], FR)
            kTi = persist.tile([P, NDK, S], FR)

            # [mr | mi | -mi] per (h, a), col = (h*NDK + a)*3 + comp
            # On the Act queue: its SEQ only reaches this issue after real
            # work, keeping the single-slot DMA engine free at t=0 for the
            # weight/hidden loads the first matmuls block on.
            # (shipped pre-transposed: the old "h a c p -> p (h a c)"
            # rearrange was 54 four-byte descriptors per partition, a ~3us
            # descriptor-bound DMA blocking the startup stream)
            mixv = vsmall.tile([P, HPC * NDK * 3], FP)
            nc.scalar.dma_start(mixv, mixv_d[:])
            # per-head context rhs: [vr_h | vi_h | 1]
            vaug = [vsmall.tile([P, NT, 2 * DH + 1], HF, tag=f"vaug{h}",
                                name=f"vaug{h}")
                    for h in range(HPC)]
            for h in range(HPC):
                nc.vector.memset(vaug[h][:, :, 2 * DH], 1.0)
            # (cbr/8 | cbi/8) per head, flattened: col = tt*2*HPC + (0|HPC) + h
            cb8 = vsmall.tile([P, NT * 2 * HPC], FP)

            # ---- phase P: projections -----------------------------------
            # hs holds hr+hi for the projection Karatsuba t3 streams; once
            # q/k are done it is overwritten with kTs = kr+ki (the score
            # Karatsuba stationary), so its 24KB/partition is reused and the
            # adds hide under the v/cb matmul window.
            ctx_hsp = tc.tile_pool(name="hsp", bufs=1)
            hsp = ctx_hsp.__enter__()
            hs = hsp.tile([P, ND, S], FR, tag="hs")
            with tc.tile_pool(name="hload", bufs=1) as hload:
                hr = hload.tile([P, ND, S], FR, tag="hr")
                hi = hload.tile([P, ND, S], FR, tag="hi")

                def _emit_hidden_half(j):
                    # Half-plane hidden DMAs on the same HWDGE queue as the
                    # weights, emitted between weight loads so the single
                    # DMA slot serves the matmul chains in consumption
                    # order. hs = hr+hi is built by the otherwise-idle Pool
                    # engine right behind the DMA stream instead of being
                    # shipped from host.
                    ssl = slice(j * SWP, (j + 1) * SWP)
                    for d in range(ND):
                        nc.sync.dma_start(hi[:, d, ssl], hTi_d[d, :, ssl])
                        nc.sync.dma_start(hr[:, d, ssl], hTr_d[d, :, ssl])
                    for d in range(ND):
                        nc.gpsimd.tensor_add(hs[:, d, ssl], hr[:, d, ssl],
                                             hi[:, d, ssl])

                # q/k projections: out[n, s] = sum_d W[d, n] * hT[d, s]
                ctx_vw = tc.tile_pool(name="vwides", bufs=1)
                vwides = ctx_vw.__enter__()
                with (
                    tc.tile_pool(name="wload", bufs=2) as wload,
                    tc.tile_pool(name="wsload", bufs=1) as wsload,
                    tc.tile_pool(name="pp23", bufs=1, space="PSUM") as pp23,
                    tc.tile_pool(name="pp1", bufs=2, space="PSUM") as pp1,
                    tc.tile_pool(name="stage", bufs=2) as stage,
                    tc.tile_pool(name="stagex", bufs=1) as stagex,
                ):
                    for (w_d, dst_r, dst_i, gname) in (
                        (wk_d, kTr, kTi, "k"),
                        (wq_d, qTr, qTi, "q"),
                    ):
                        def _load_w(nt, w_d=w_d):
                            # wi first: the t2 chains consume it first
                            nsl_ = slice(nt * P, (nt + 1) * P)
                            wi_ = wload.tile([P, ND, P], FR, tag="wi")
                            wr_ = wload.tile([P, ND, P], FR, tag="wr")
                            nc.sync.dma_start(
                                wi_,
                                w_d["i"][:, :, nsl_].rearrange("a p c -> p a c"))
                            nc.sync.dma_start(
                                wr_,
                                w_d["r"][:, :, nsl_].rearrange("a p c -> p a c"))
                            return wi_, wr_

                        # 2-deep weight prefetch: nt0 AND nt1 weights go on
                        # the (in-order, single-slot) DMA path before the
                        # 6.3MB of hidden, so nt1's chains don't stall
                        # behind the hidden stream.
                        pend = {0: _load_w(0), 1: _load_w(1)}
                        if gname == "k":
                            # all hidden DMAs must precede the first chains
                            # in program order (deps are tracked in program
                            # order)
                            _emit_hidden_half(0)
                            _emit_hidden_half(1)
                        for nt in range(NDK):
                            t1 = [pp1.tile([P, SWP], FP, tag=f"t1s{j}",
                                           name=f"{gname}t1s{j}n{nt}")
                                  for j in range(2)]
                            t2 = [pp23.tile([P, SWP], FP, tag=f"t2s{j}",
                                            name=f"{gname}t2s{j}n{nt}")
                                  for j in range(2)]
                            t3 = [pp23.tile([P, SWP], FP, tag=f"t3s{j}",
                                            name=f"{gname}t3s{j}n{nt}")
                                  for j in range(2)]
                            wi, wr = pend.pop(nt)
                            if nt + 2 < NDK:
                                pend[nt + 2] = _load_w(nt + 2)
                            ws = wsload.tile([P, ND, P], FR, tag="ws")
                            nc.vector.tensor_add(ws, wr, wi)
                            # t2 chains first so their stops land earliest
                            # and the Act stage copy overlaps the t1/t3 tails
                            for src, tt_, htile in ((wi, t2, hi), (wr, t1, hr),
                                                    (ws, t3, hs)):
                                for j in range(2):
                                    ssl = slice(j * SWP, (j + 1) * SWP)
                                    for d in range(ND):
                                        nc.tensor.matmul(
                                            tt_[j], src[:, d], htile[:, d, ssl],
                                            start=d == 0, stop=d == ND - 1)
                            for j in range(2):
                                ssl = slice(j * SWP, (j + 1) * SWP)
                                s2 = stage.tile([P, SWP], FP, tag="s2")
                                nc.scalar.activation(s2, t2[j], AF.Copy)
                                # real = t1 - t2 ; imag = (t3 - t2) - t1
                                nc.vector.tensor_sub(dst_r[:, nt, ssl], t1[j], s2)
                                x = stagex.tile([P, SWP], FP, tag="x")
                                nc.vector.tensor_sub(x, t3[j], s2)
                                nc.vector.tensor_sub(dst_i[:, nt, ssl], x, t1[j])

                        if gname == "q":
                            # issue the v/cb-pass loads now, on the Act queue
                            # (reached mid-q): the 1.2MB wvcb transfer stays
                            # clear of the startup stream yet lands before
                            # the v/cb pass consumes it
                            bvec_sb = vwides.tile([1, P + WVCB], FR)
                            nc.scalar.dma_start(bvec_sb, bvec_d[:])
                            wvcb_sb = vwides.tile([P, ND, WVCB], FR)
                            nc.scalar.dma_start(
                                wvcb_sb, wvcb_d[:].rearrange("a p c -> p a c"))

                    # kTs = kr + ki, overwriting hs in place (program order
                    # is past q's t3 reads here). On the Pool engine so DVE
                    # reaches the first mq block during the v/cb pass.
                    for nt in range(NDK):
                        nc.gpsimd.tensor_add(hs[:, nt], kTr[:, nt], kTi[:, nt])



                # v / cb projections: [tok, c] = sum_d hT[d, tok] * Wbig[d, c]
                with (
                    tc.tile_pool(name="pv", bufs=2, space="PSUM") as pv,
                    tc.tile_pool(name="cbtmp", bufs=2) as cbtmp,
                    tc.tile_pool(name="vstage", bufs=4) as vstage,
                ):
                    for tt in range(NT):
                        tsl = slice(tt * P, (tt + 1) * P)
                        psA = pv.tile([P, WVCB], FP, tag="psA")
                        psB = pv.tile([P, WVCB], FP, tag="psB")
                        # bias row (bv | 0) into psA first (fewest deps first)
                        nc.tensor.matmul(psA, bvec_sb[:, :P], bvec_sb[:, P:],
                                         start=True, stop=False)
                        for d in range(ND):
                            nc.tensor.matmul(psA, hr[:, d, tsl], wvcb_sb[:, d],
                                             start=False, stop=(d == ND - 1))
                            nc.tensor.matmul(psB, hi[:, d, tsl], wvcb_sb[:, d],
                                             start=(d == 0), stop=(d == ND - 1))
                        # Stage both PSUM tiles via Act, then combine on the
                        # Pool engine: DVE stays free so its in-order stream
                        # reaches the first mq block during this pass.
                        sA = vstage.tile([P, WVCB], FP, tag="sA")
                        sB = vstage.tile([P, WVCB], FP, tag="sB")
                        nc.scalar.activation(sA, psA, AF.Copy)
                        nc.scalar.activation(sB, psB, AF.Copy)
                        for h in range(HPC):
                            c0 = h * DH
                            # vr_h = A[vr] - B[vi];  vi_h = A[vi] + B[vr]
                            nc.gpsimd.tensor_sub(vaug[h][:, tt, 0:DH],
                                                 sA[:, c0:c0 + DH],
                                                 sB[:, VC + c0:VC + c0 + DH])
                            nc.gpsimd.tensor_add(vaug[h][:, tt, DH:2 * DH],
                                                 sA[:, VC + c0:VC + c0 + DH],
                                                 sB[:, c0:c0 + DH])
                        # cb8: (A[cbr] - B[cbi])/8 , (A[cbi] + B[cbr])/8
                        # (tiny subs on DVE — on Pool they'd sit behind the
                        # long kTs adds and head-of-line-block Act's cb8
                        # copies, stalling the whole v/cb pass)
                        tr = cbtmp.tile([P, HPC], FP, tag="tr")
                        ti = cbtmp.tile([P, HPC], FP, tag="ti")
                        nc.vector.tensor_sub(tr, sA[:, 2 * VC:2 * VC + HPC],
                                             sB[:, 2 * VC + HPC:2 * VC + 2 * HPC])
                        nc.vector.tensor_add(ti, sA[:, 2 * VC + HPC:2 * VC + 2 * HPC],
                                             sB[:, 2 * VC:2 * VC + HPC])
                        # (tensor_scalar is not a legal Pool-engine opcode;
                        # these tiny muls go to Act as scaled copies)
                        cbc = tt * 2 * HPC
                        nc.scalar.activation(cb8[:, cbc:cbc + HPC], tr,
                                             AF.Copy, scale=0.125)
                        nc.scalar.activation(cb8[:, cbc + HPC:cbc + 2 * HPC], ti,
                                             AF.Copy, scale=0.125)
                ctx_vw.__exit__(None, None, None)

            # ---- phase S: per-head scores -> softmax -> context ---------
            kTs = hs  # computed in place at the end of phase P
            with (
                tc.tile_pool(name="mqp", bufs=2) as mqp,
                tc.tile_pool(name="ep", bufs=2) as ep,
                tc.tile_pool(name="etmp", bufs=4) as etmp,
                tc.tile_pool(name="psc", bufs=2, space="PSUM") as psc,
                tc.tile_pool(name="pctx", bufs=1, space="PSUM") as pctx,
                tc.tile_pool(name="ctxs", bufs=4) as ctxs,
            ):
                for h in range(HPC):
                    for ss in range(NSS):
                        ssl = slice(ss * SWS, (ss + 1) * SWS)
                        mqr = mqp.tile([P, NDK, SWS], FR, tag="mqr")
                        mqi = mqp.tile([P, NDK, SWS], FR, tag="mqi")
                        mqs = mqp.tile([P, NDK, SWS], FR, tag="mqs")
                        # grouped by kind: mqr completes first so the t1
                        # matmul group can start while mqi/mqs still build
                        for a in range(NDK):
                            mbase = (h * NDK + a) * 3
                            mr = mixv[:, mbase:mbase + 1]
                            min_ = mixv[:, mbase + 2:mbase + 3]
                            # mqr = qTr*mr - qTi*mi
                            nc.vector.tensor_scalar_mul(mqr[:, a], qTr[:, a, ssl], mr)
                            nc.vector.scalar_tensor_tensor(
                                mqr[:, a], qTi[:, a, ssl], min_, mqr[:, a],
                                op0=OP.mult, op1=OP.add)
                        for a in range(NDK):
                            mbase = (h * NDK + a) * 3
                            mr = mixv[:, mbase:mbase + 1]
                            mi = mixv[:, mbase + 1:mbase + 2]
                            # mqi = qTr*mi + qTi*mr
                            nc.vector.tensor_scalar_mul(mqi[:, a], qTr[:, a, ssl], mi)
                            nc.vector.scalar_tensor_tensor(
                                mqi[:, a], qTi[:, a, ssl], mr, mqi[:, a],
                                op0=OP.mult, op1=OP.add)
                        for a in range(NDK):
                            # Pool engine: keeps DVE (the hotter engine in
                            # phase S) off the mqs adds
                            nc.gpsimd.tensor_add(mqs[:, a], mqr[:, a], mqi[:, a])

                        Er = ep.tile([P, NT, SWS], HF, tag="Er")
                        Ei = ep.tile([P, NT, SWS], HF, tag="Ei")
                        for tt in range(NT):
                            tsl = slice(tt * P, (tt + 1) * P)
                            t1 = psc.tile([P, SWS], FP, tag="t1")
                            t2 = psc.tile([P, SWS], FP, tag="t2")
                            t3 = psc.tile([P, SWS], FP, tag="t3")
                            # grouped t1 -> t2 -> t3 so e1/e4 overlap t2/t3
                            for a in range(NDK):
                                nc.tensor.matmul(t1, kTr[:, a, tsl], mqr[:, a],
                                                 start=a == 0, stop=a == NDK - 1)
                            for a in range(NDK):
                                nc.tensor.matmul(t2, kTi[:, a, tsl], mqi[:, a],
                                                 start=a == 0, stop=a == NDK - 1)
                            for a in range(NDK):
                                nc.tensor.matmul(t3, kTs[:, a, tsl], mqs[:, a],
                                                 start=a == 0, stop=a == NDK - 1)
                            # Er = exp((t1-t2+cbr)/8) = e1*e2
                            # Ei = exp((t3-t1-t2+cbi)/8) = e3*e2*e4
                            cbc = tt * 2 * HPC
                            e1 = etmp.tile([P, SWS], HF, tag="e1")
                            e2 = etmp.tile([P, SWS], HF, tag="e2")
                            e3 = etmp.tile([P, SWS], HF, tag="e3")
                            e4 = etmp.tile([P, SWS], HF, tag="e4")
                            nc.scalar.activation(
                                e1, t1, AF.Exp,
                                bias=cb8[:, cbc + h:cbc + h + 1], scale=0.125)
                            nc.scalar.activation(e4, t1, AF.Exp, scale=-0.125)
                            nc.scalar.activation(e2, t2, AF.Exp, scale=-0.125)
                            nc.scalar.activation(
                                e3, t3, AF.Exp,
                                bias=cb8[:, cbc + HPC + h:cbc + HPC + h + 1],
                                scale=0.125)
                            nc.vector.tensor_mul(Er[:, tt], e1, e2)
                            m1 = etmp.tile([P, SWS], HF, tag="m1")
                            nc.vector.tensor_mul(m1, e3, e4)
                            nc.vector.tensor_mul(Ei[:, tt], m1, e2)

                        # context: for each 128-row block of queries
                        for sj in range(SWS // P):
                            st_idx = ss * (SWS // P) + sj
                            qsl = slice(sj * P, (sj + 1) * P)
                            pcA = pctx.tile([P, 2 * DH + 1], FP, tag="pcA")
                            pcB = pctx.tile([P, 2 * DH + 1], FP, tag="pcB")
                            for tt in range(NT):
                                st, sp = tt == 0, tt == NT - 1
                                nc.tensor.matmul(pcA, Er[:, tt, qsl], vaug[h][:, tt],
                                                 start=st, stop=sp)
                                nc.tensor.matmul(pcB, Ei[:, tt, qsl], vaug[h][:, tt],
                                                 start=st, stop=sp)
                            # Act stages the tiny context tiles out of PSUM
                            # (freeing the banks for the next sj) and takes
                            # the reciprocals; the Pool engine does the
                            # complex combine. DVE stays out of the ctx
                            # readout entirely.
                            sA2 = ctxs.tile([P, 2 * DH + 1], FP, tag="sA2")
                            sB2 = ctxs.tile([P, 2 * DH + 1], FP, tag="sB2")
                            nc.scalar.activation(sA2, pcA, AF.Copy)
                            nc.scalar.activation(sB2, pcB, AF.Copy)
                            rr = ctxs.tile([P, 1], FP, tag="rr")
                            ri = ctxs.tile([P, 1], FP, tag="ri")
                            nc.vector.reciprocal(rr, sA2[:, 2 * DH:2 * DH + 1])
                            nc.vector.reciprocal(ri, sB2[:, 2 * DH:2 * DH + 1])
                            # cr = A/sumr - Bvi/sumi ; ci = Avi/sumr + Bvr/sumi
                            tb = ctxs.tile([P, DH], FP, tag="tb")
                            td = ctxs.tile([P, DH], FP, tag="td")
                            cr = ctxs.tile([P, DH], FP, tag="cr")
                            ci = ctxs.tile([P, DH], FP, tag="ci")
                            nc.vector.tensor_scalar_mul(tb, sB2[:, DH:2 * DH], ri)
                            nc.vector.scalar_tensor_tensor(
                                cr, sA2[:, 0:DH], rr, tb, op0=OP.mult,
                                op1=OP.subtract)
                            nc.vector.tensor_scalar_mul(td, sB2[:, 0:DH], ri)
                            nc.vector.scalar_tensor_tensor(
                                ci, sA2[:, DH:2 * DH], rr, td, op0=OP.mult,
                                op1=OP.add)
                            nc.sync.dma_start(out_d[0, h, st_idx], cr)
                            nc.sync.dma_start(out_d[1, h, st_idx], ci)

            ctx_hsp.__exit__(None, None, None)

    if split_waits:
        _split_multi_waits(nc)
    return nc


def _prep_core_inputs(inputs, core):
    b = core // (N_CORES // B)
    hb = core % (N_CORES // B)
    heads = list(range(hb * HPC, (hb + 1) * HPC))
    cols = slice(hb * VC, (hb + 1) * VC)

    f32 = lambda x: np.ascontiguousarray(np.asarray(x, dtype=np.float32))
    hr = f32(inputs["hidden_r"][b]).T    # [D, S]
    hi = f32(inputs["hidden_i"][b]).T

    wv = np.concatenate(
        [f32(inputs["Wv_r"])[:, cols], f32(inputs["Wv_i"])[:, cols],
         f32(inputs["Wcb_r"])[:, heads], f32(inputs["Wcb_i"])[:, heads]], axis=1)
    bv = np.concatenate(
        [np.ones(P, np.float32),
         f32(inputs["bv_r"])[cols], f32(inputs["bv_i"])[cols],
         np.zeros(2 * HPC, np.float32)])

    mr = f32(inputs["mix_r"])[heads]     # [HPC, DK]
    mi = f32(inputs["mix_i"])[heads]
    mixv = np.stack([mr, mi, -mi], axis=-1)  # [HPC, DK, 3]

    c = np.ascontiguousarray
    return {
        "hTr": c(hr.reshape(ND, P, S)),
        "hTi": c(hi.reshape(ND, P, S)),
        "wqr": c(f32(inputs["Wq_r"]).reshape(ND, P, DK)),
        "wqi": c(f32(inputs["Wq_i"]).reshape(ND, P, DK)),
        "wkr": c(f32(inputs["Wk_r"]).reshape(ND, P, DK)),
        "wki": c(f32(inputs["Wk_i"]).reshape(ND, P, DK)),
        "wvcb": c(wv.reshape(ND, P, WVCB)),
        "bvec": c(bv.reshape(1, P + WVCB)),
        "mixv": c(mixv.reshape(HPC, NDK, P, 3).transpose(2, 0, 1, 3)
                  .reshape(P, HPC * NDK * 3)),
    }


def kernel(**inputs):
    global _compiled, LAST_RESULTS
    if _compiled is None:
        _compiled = _build()
    nc = _compiled

    in_maps = [_prep_core_inputs(inputs, c) for c in range(N_CORES)]
    res = run_bass_kernel_spmd(nc, in_maps, core_ids=list(range(N_CORES)),
                               trace=TRACE)
    LAST_RESULTS = res

    out = np.zeros((2, B, S, DV), np.float32)
    for core in range(N_CORES):
        b = core // (N_CORES // B)
        hb = core % (N_CORES // B)
        oc = res.results[core]["out"]  # [2, HPC, NT, P, DH]
        for j in range(HPC):
            h = hb * HPC + j
            out[:, b, :, h * DH:(h + 1) * DH] = oc[:, j].reshape(2, S, DH)
    return out


# revision 59
# speedup vs baseline: 1.0102x; 1.0003x over previous
"""CollaborativeAttention (complex-valued, per-head mixed queries) on 8 trn2 cores.

Sharding: B*H = 24 (batch, head) units -> 3 heads per core.
  core c: batch b = c // 4, head block hb = c % 4 -> heads [3*hb, 3*hb+2].
Each core computes q/k projections for its batch (replicated within the
4-core batch group), v/cb projections for its head block only, then
scores+softmax+context for its 3 heads.

Karatsuba everywhere: a complex matmul (A_r+iA_i)(B_r+iB_i) is computed as
three real accumulation streams t1=A_r B_r, t2=A_i B_i, t3=(A_r+A_i)(B_r+B_i)
with real = t1-t2, imag = t3-t1-t2 — 25% less PE time than the 4-stream form.

 - Projections: hidden is fully resident in SBUF; hs=hr+hi is built by the
   Pool engine right behind the DMA stream, ws=wr+wi per weight tile by DVE,
   so every HBM byte is fetched exactly once on the (single-slot) DMA path,
   ordered so the matmul chains chase the stream. Combines: s2=copy(t2) on
   Act, then real = t1-s2, x = t3-s2, imag = x-t1 on DVE (one PSUM operand
   per op). kTs = kr+ki (the score Karatsuba stationary) overwrites the hs
   tile on Pool during the v/cb window.
 - Scores run transposed, sT[t, s] (key pos on partitions), in fp32r (full
   PE rate at >=256 moving rows). The exp is FACTORED instead of combined:
     Er = exp((t1-t2+cbr)/8) = exp((t1+cbr)/8) * exp(-t2/8) = e1*e2
     Ei = exp((t3-t1-t2+cbi)/8) = e3*e2*e4,  e4 = exp(-t1/8)
   so the Act engine reads each PSUM bank directly (4 exps, content bias
   fused as a per-partition ACT bias) and DVE does 3 cheap fp16 multiplies;
   no PSUM->SBUF staging pass is needed.
 - mixed queries: mqr/mqi via DVE tensor_scalar ops with per-partition
   mixing scalars; mqs = mqr+mqi on Pool. Double-buffered so the next
   (head, s-slice) block's DVE/Pool work overlaps this block's PE.
 - context matmul (moving dim 129 < 256) runs in fp16: probs and the
   per-head value matrix [vr | vi | 1] (ones column = softmax denominator);
   normalization happens on the tiny [128, 64] context tiles after an Act
   stage-out of PSUM (Pool has no PSUM port; tensor_scalar is not a legal
   Pool opcode — engine placement is ISA-constrained).
This walrus build encodes at most one sync-wait per instruction, so a
post-pass (_split_multi_waits) peels extra waits onto NoOps.
"""

import sys

for _p in ("/opt/trn_rl_repo", "/root/.axon_site", "/root/.axon_site/_ro/trn_rl_repo",
           "/root/.axon_site/_ro/pypackages"):
    if _p not in sys.path:
        sys.path.append(_p)

import numpy as np

import concourse.bass as bass
import concourse.mybir as mybir
import concourse.tile as tile
from concourse.bass_utils import run_bass_kernel_spmd

B, S, D, H = 2, 1024, 768, 12
DK = DV = 768
DH = DV // H          # 64 per-head value dim
HPC = 3               # heads per core
N_CORES = 8
P = 128
ND = D // P           # 6 d-tiles (contraction)
NDK = DK // P         # 6 output n-tiles for q/k
NT = S // P           # 8 token tiles
SWP = 512             # s-half width in projection PSUM tiles
SWS = 256             # s-slice width for scores
NSS = S // SWS        # 4 score s-slices
VC = HPC * DH         # 192 value cols per core
WVCB = 2 * VC + 2 * HPC  # 390: [Wv_r | Wv_i | Wcb_r | Wcb_i] cols

FP = mybir.dt.float32
FR = mybir.dt.float32r
HF = mybir.dt.float16
AF = mybir.ActivationFunctionType
OP = mybir.AluOpType

TRACE = False
LAST_RESULTS = None

_compiled = None


def _split_multi_waits(nc):
    """The walrus build here encodes at most ONE sync-wait per instruction
    ("Too many sync wait commands" in setupSyncWait otherwise). Tile freely
    emits several. Split the extras onto single-wait NoOps that precede the
    instruction in the same engine stream."""
    for fn in nc.m.functions:
        for bb in fn.blocks:
            out = []
            for ins in bb.instructions:
                si = ins.sync_info
                if si is not None and len(si.on_wait) > 1:
                    waits = list(si.on_wait)
                    for j, w in enumerate(waits[:-1]):
                        nop = mybir.InstNoOp(name=f"{ins.name}-ws{j}",
                                             ins=[], outs=[])
                        nop.engine = ins.engine
                        nop.sync_info = mybir.SyncInfo(on_wait=[w], on_update=[])
                        out.append(nop)
                    ins.sync_info = mybir.SyncInfo(on_wait=[waits[-1]],
                                                   on_update=list(si.on_update))
                out.append(ins)
            bb.instructions = out


def _build(split_waits=True):
    """Build the SPMD Bass program (identical on all 8 cores)."""
    nc = bass.Bass(trn_type="TRN2")

    hTr_d = nc.dram_tensor("hTr", [ND, P, S], FR, kind="ExternalInput")
    hTi_d = nc.dram_tensor("hTi", [ND, P, S], FR, kind="ExternalInput")
    wq_d = {c: nc.dram_tensor(f"wq{c}", [ND, P, DK], FR, kind="ExternalInput")
            for c in "ri"}
    wk_d = {c: nc.dram_tensor(f"wk{c}", [ND, P, DK], FR, kind="ExternalInput")
            for c in "ri"}
    wvcb_d = nc.dram_tensor("wvcb", [ND, P, WVCB], FR, kind="ExternalInput")
    bvec_d = nc.dram_tensor("bvec", [1, P + WVCB], FR, kind="ExternalInput")
    mixv_d = nc.dram_tensor("mixv", [P, HPC * NDK * 3], FP, kind="ExternalInput")
    out_d = nc.dram_tensor("out", [2, HPC, NT, P, DH], FP, kind="ExternalOutput")

    with tile.TileContext(nc) as tc:
        with (
            tc.tile_pool(name="persist", bufs=1) as persist,
            tc.tile_pool(name="vsmall", bufs=1) as vsmall,
        ):
            # ---- persistent tensors -------------------------------------
            # (kTs reuses the hs tile below — computed in place at the end
            # of phase P so no extra 24KB/partition is needed)
            qTr = persist.tile([P, NDK, S], FP)
            qTi = persist.tile([P, NDK, S], FP)
            kTr = persist.tile([P, NDK, S], FR)
            kTi = persist.tile([P, NDK, S], FR)

            # [mr | mi | -mi] per (h, a), col = (h*NDK + a)*3 + comp
            # On the Act queue: its SEQ only reaches this issue after real
            # work, keeping the single-slot DMA engine free at t=0 for the
            # weight/hidden loads the first matmuls block on.
            # (shipped pre-transposed: the old "h a c p -> p (h a c)"
            # rearrange was 54 four-byte descriptors per partition, a ~3us
            # descriptor-bound DMA blocking the startup stream)
            mixv = vsmall.tile([P, HPC * NDK * 3], FP)
            nc.scalar.dma_start(mixv, mixv_d[:])
            # per-head context rhs: [vr_h | vi_h | 1]
            vaug = [vsmall.tile([P, NT, 2 * DH + 1], HF, tag=f"vaug{h}",
                                name=f"vaug{h}")
                    for h in range(HPC)]
            for h in range(HPC):
                nc.vector.memset(vaug[h][:, :, 2 * DH], 1.0)
            # (cbr/8 | cbi/8) per head, flattened: col = tt*2*HPC + (0|HPC) + h
            cb8 = vsmall.tile([P, NT * 2 * HPC], FP)

            # ---- phase P: projections -----------------------------------
            # hs holds hr+hi for the projection Karatsuba t3 streams; once
            # q/k are done it is overwritten with kTs = kr+ki (the score
            # Karatsuba stationary), so its 24KB/partition is reused and the
            # adds hide under the v/cb matmul window.
            ctx_hsp = tc.tile_pool(name="hsp", bufs=1)
            hsp = ctx_hsp.__enter__()
            hs = hsp.tile([P, ND, S], FR, tag="hs")
            with tc.tile_pool(name="hload", bufs=1) as hload:
                hr = hload.tile([P, ND, S], FR, tag="hr")
                hi = hload.tile([P, ND, S], FR, tag="hi")

                def _emit_hidden_half(j):
                    # Half-plane hidden DMAs on the same HWDGE queue as the
                    # weights, emitted between weight loads so the single
                    # DMA slot serves the matmul chains in consumption
                    # order. hs = hr+hi is built by the otherwise-idle Pool
                    # engine right behind the DMA stream instead of being
                    # shipped from host.
                    ssl = slice(j * SWP, (j + 1) * SWP)
                    for d in range(ND):
                        nc.sync.dma_start(hi[:, d, ssl], hTi_d[d, :, ssl])
                        nc.sync.dma_start(hr[:, d, ssl], hTr_d[d, :, ssl])
                    for d in range(ND):
                        nc.gpsimd.tensor_add(hs[:, d, ssl], hr[:, d, ssl],
                                             hi[:, d, ssl])

                # q/k projections: out[n, s] = sum_d W[d, n] * hT[d, s]
                ctx_vw = tc.tile_pool(name="vwides", bufs=1)
                vwides = ctx_vw.__enter__()
                with (
                    tc.tile_pool(name="wload", bufs=2) as wload,
                    tc.tile_pool(name="wsload", bufs=1) as wsload,
                    tc.tile_pool(name="pp23", bufs=1, space="PSUM") as pp23,
                    tc.tile_pool(name="pp1", bufs=2, space="PSUM") as pp1,
                    tc.tile_pool(name="stage", bufs=2) as stage,
                    tc.tile_pool(name="stagex", bufs=1) as stagex,
                ):
                    for (w_d, dst_r, dst_i, gname) in (
                        (wk_d, kTr, kTi, "k"),
                        (wq_d, qTr, qTi, "q"),
                    ):
                        def _load_w(nt, w_d=w_d):
                            # wi first: the t2 chains consume it first
                            nsl_ = slice(nt * P, (nt + 1) * P)
                            wi_ = wload.tile([P, ND, P], FR, tag="wi")
                            wr_ = wload.tile([P, ND, P], FR, tag="wr")
                            nc.sync.dma_start(
                                wi_,
                                w_d["i"][:, :, nsl_].rearrange("a p c -> p a c"))
                            nc.sync.dma_start(
                                wr_,
                                w_d["r"][:, :, nsl_].rearrange("a p c -> p a c"))
                            return wi_, wr_

                        # 2-deep weight prefetch: nt0 AND nt1 weights go on
                        # the (in-order, single-slot) DMA path before the
                        # 6.3MB of hidden, so nt1's chains don't stall
                        # behind the hidden stream.
                        pend = {0: _load_w(0), 1: _load_w(1)}
                        if gname == "k":
                            # all hidden DMAs must precede the first chains
                            # in program order (deps are tracked in program
                            # order)
                            _emit_hidden_half(0)
                            _emit_hidden_half(1)
                            # wvcb via the Pool SWDGE path, queued after the
                            # hs adds: Pool executes DMAs in-order on the
                            # engine, so this 1.2MB transfer provably cannot
                            # wedge into the startup stream (HWDGE-queue
                            # SEQs run ahead of blocked instructions and
                            # would issue it at t~5us otherwise)
                            wvcb_sb = vwides.tile([P, ND, WVCB], FR)
                            nc.gpsimd.dma_start(
                                wvcb_sb, wvcb_d[:].rearrange("a p c -> p a c"))
                        for nt in range(NDK):
                            t1 = [pp1.tile([P, SWP], FP, tag=f"t1s{j}",
                                           name=f"{gname}t1s{j}n{nt}")
                                  for j in range(2)]
                            t2 = [pp23.tile([P, SWP], FP, tag=f"t2s{j}",
                                            name=f"{gname}t2s{j}n{nt}")
                                  for j in range(2)]
                            t3 = [pp23.tile([P, SWP], FP, tag=f"t3s{j}",
                                            name=f"{gname}t3s{j}n{nt}")
                                  for j in range(2)]
                            wi, wr = pend.pop(nt)
                            if nt + 2 < NDK:
                                pend[nt + 2] = _load_w(nt + 2)
                            ws = wsload.tile([P, ND, P], FR, tag="ws")
                            nc.vector.tensor_add(ws, wr, wi)
                            # t2 chains first so their stops land earliest
                            # and the Act stage copy overlaps the t1/t3 tails
                            for src, tt_, htile in ((wi, t2, hi), (wr, t1, hr),
                                                    (ws, t3, hs)):
                                for j in range(2):
                                    ssl = slice(j * SWP, (j + 1) * SWP)
                                    for d in range(ND):
                                        nc.tensor.matmul(
                                            tt_[j], src[:, d], htile[:, d, ssl],
                                            start=d == 0, stop=d == ND - 1)
                            for j in range(2):
                                ssl = slice(j * SWP, (j + 1) * SWP)
                                s2 = stage.tile([P, SWP], FP, tag="s2")
                                nc.scalar.activation(s2, t2[j], AF.Copy)
                                # real = t1 - t2 ; imag = (t3 - t2) - t1
                                nc.vector.tensor_sub(dst_r[:, nt, ssl], t1[j], s2)
                                x = stagex.tile([P, SWP], FP, tag="x")
                                nc.vector.tensor_sub(x, t3[j], s2)
                                nc.vector.tensor_sub(dst_i[:, nt, ssl], x, t1[j])

                        if gname == "q":
                            bvec_sb = vwides.tile([1, P + WVCB], FR)
                            nc.scalar.dma_start(bvec_sb, bvec_d[:])

                    # kTs = kr + ki, overwriting hs in place (program order
                    # is past q's t3 reads here). On the Pool engine so DVE
                    # reaches the first mq block during the v/cb pass.
                    for nt in range(NDK):
                        nc.gpsimd.tensor_add(hs[:, nt], kTr[:, nt], kTi[:, nt])



                # v / cb projections: [tok, c] = sum_d hT[d, tok] * Wbig[d, c]
                with (
                    tc.tile_pool(name="pv", bufs=2, space="PSUM") as pv,
                    tc.tile_pool(name="cbtmp", bufs=2) as cbtmp,
                    tc.tile_pool(name="vstage", bufs=4) as vstage,
                ):
                    for tt in range(NT):
                        tsl = slice(tt * P, (tt + 1) * P)
                        psA = pv.tile([P, WVCB], FP, tag="psA")
                        psB = pv.tile([P, WVCB], FP, tag="psB")
                        # bias row (bv | 0) into psA first (fewest deps first)
                        nc.tensor.matmul(psA, bvec_sb[:, :P], bvec_sb[:, P:],
                                         start=True, stop=False)
                        for d in range(ND):
                            nc.tensor.matmul(psA, hr[:, d, tsl], wvcb_sb[:, d],
                                             start=False, stop=(d == ND - 1))
                            nc.tensor.matmul(psB, hi[:, d, tsl], wvcb_sb[:, d],
                                             start=(d == 0), stop=(d == ND - 1))
                        # Stage both PSUM tiles via Act, then combine on the
                        # Pool engine: DVE stays free so its in-order stream
                        # reaches the first mq block during this pass.
                        sA = vstage.tile([P, WVCB], FP, tag="sA")
                        sB = vstage.tile([P, WVCB], FP, tag="sB")
                        nc.scalar.activation(sA, psA, AF.Copy)
                        nc.scalar.activation(sB, psB, AF.Copy)
                        for h in range(HPC):
                            c0 = h * DH
                            # vr_h = A[vr] - B[vi];  vi_h = A[vi] + B[vr]
                            nc.gpsimd.tensor_sub(vaug[h][:, tt, 0:DH],
                                                 sA[:, c0:c0 + DH],
                                                 sB[:, VC + c0:VC + c0 + DH])
                            nc.gpsimd.tensor_add(vaug[h][:, tt, DH:2 * DH],
                                                 sA[:, VC + c0:VC + c0 + DH],
                                                 sB[:, c0:c0 + DH])
                        # cb8: (A[cbr] - B[cbi])/8 , (A[cbi] + B[cbr])/8
                        # (tiny subs on DVE — on Pool they'd sit behind the
                        # long kTs adds and head-of-line-block Act's cb8
                        # copies, stalling the whole v/cb pass)
                        tr = cbtmp.tile([P, HPC], FP, tag="tr")
                        ti = cbtmp.tile([P, HPC], FP, tag="ti")
                        nc.vector.tensor_sub(tr, sA[:, 2 * VC:2 * VC + HPC],
                                             sB[:, 2 * VC + HPC:2 * VC + 2 * HPC])
                        nc.vector.tensor_add(ti, sA[:, 2 * VC + HPC:2 * VC + 2 * HPC],
                                             sB[:, 2 * VC:2 * VC + HPC])
                        # (tensor_scalar is not a legal Pool-engine opcode;
                        # these tiny muls go to Act as scaled copies)
                        cbc = tt * 2 * HPC
                        nc.scalar.activation(cb8[:, cbc:cbc + HPC], tr,
                                             AF.Copy, scale=0.125)
                        nc.scalar.activation(cb8[:, cbc + HPC:cbc + 2 * HPC], ti,
                                             AF.Copy, scale=0.125)
                ctx_vw.__exit__(None, None, None)

            # ---- phase S: per-head scores -> softmax -> context ---------
            kTs = hs  # computed in place at the end of phase P
            with (
                tc.tile_pool(name="mqp", bufs=2) as mqp,
                tc.tile_pool(name="ep", bufs=2) as ep,
                tc.tile_pool(name="etmp", bufs=4) as etmp,
                tc.tile_pool(name="psc", bufs=2, space="PSUM") as psc,
                tc.tile_pool(name="pctx", bufs=1, space="PSUM") as pctx,
                tc.tile_pool(name="ctxs", bufs=4) as ctxs,
            ):
                for h in range(HPC):
                    for ss in range(NSS):
                        ssl = slice(ss * SWS, (ss + 1) * SWS)
                        mqr = mqp.tile([P, NDK, SWS], FR, tag="mqr")
                        mqi = mqp.tile([P, NDK, SWS], FR, tag="mqi")
                        mqs = mqp.tile([P, NDK, SWS], FR, tag="mqs")
                        # grouped by kind: mqr completes first so the t1
                        # matmul group can start while mqi/mqs still build
                        for a in range(NDK):
                            mbase = (h * NDK + a) * 3
                            mr = mixv[:, mbase:mbase + 1]
                            min_ = mixv[:, mbase + 2:mbase + 3]
                            # mqr = qTr*mr - qTi*mi
                            nc.vector.tensor_scalar_mul(mqr[:, a], qTr[:, a, ssl], mr)
                            nc.vector.scalar_tensor_tensor(
                                mqr[:, a], qTi[:, a, ssl], min_, mqr[:, a],
                                op0=OP.mult, op1=OP.add)
                        for a in range(NDK):
                            mbase = (h * NDK + a) * 3
                            mr = mixv[:, mbase:mbase + 1]
                            mi = mixv[:, mbase + 1:mbase + 2]
                            # mqi = qTr*mi + qTi*mr
                            nc.vector.tensor_scalar_mul(mqi[:, a], qTr[:, a, ssl], mi)
                            nc.vector.scalar_tensor_tensor(
                                mqi[:, a], qTi[:, a, ssl], mr, mqi[:, a],
                                op0=OP.mult, op1=OP.add)
                        for a in range(NDK):
                            # Pool engine: keeps DVE (the hotter engine in
                            # phase S) off the mqs adds
                            nc.gpsimd.tensor_add(mqs[:, a], mqr[:, a], mqi[:, a])

                        Er = ep.tile([P, NT, SWS], HF, tag="Er")
                        Ei = ep.tile([P, NT, SWS], HF, tag="Ei")
                        for tt in range(NT):
                            tsl = slice(tt * P, (tt + 1) * P)
                            t1 = psc.tile([P, SWS], FP, tag="t1")
                            t2 = psc.tile([P, SWS], FP, tag="t2")
                            t3 = psc.tile([P, SWS], FP, tag="t3")
                            # grouped t1 -> t2 -> t3 so e1/e4 overlap t2/t3
                            for a in range(NDK):
                                nc.tensor.matmul(t1, kTr[:, a, tsl], mqr[:, a],
                                                 start=a == 0, stop=a == NDK - 1)
                            for a in range(NDK):
                                nc.tensor.matmul(t2, kTi[:, a, tsl], mqi[:, a],
                                                 start=a == 0, stop=a == NDK - 1)
                            for a in range(NDK):
                                nc.tensor.matmul(t3, kTs[:, a, tsl], mqs[:, a],
                                                 start=a == 0, stop=a == NDK - 1)
                            # Er = exp((t1-t2+cbr)/8) = e1*e2
                            # Ei = exp((t3-t1-t2+cbi)/8) = e3*e2*e4
                            cbc = tt * 2 * HPC
                            e1 = etmp.tile([P, SWS], HF, tag="e1")
                            e2 = etmp.tile([P, SWS], HF, tag="e2")
                            e3 = etmp.tile([P, SWS], HF, tag="e3")
                            e4 = etmp.tile([P, SWS], HF, tag="e4")
                            nc.scalar.activation(
                                e1, t1, AF.Exp,
                                bias=cb8[:, cbc + h:cbc + h + 1], scale=0.125)
                            nc.scalar.activation(e4, t1, AF.Exp, scale=-0.125)
                            nc.scalar.activation(e2, t2, AF.Exp, scale=-0.125)
                            nc.scalar.activation(
                                e3, t3, AF.Exp,
                                bias=cb8[:, cbc + HPC + h:cbc + HPC + h + 1],
                                scale=0.125)
                            nc.vector.tensor_mul(Er[:, tt], e1, e2)
                            m1 = etmp.tile([P, SWS], HF, tag="m1")
                            nc.vector.tensor_mul(m1, e3, e4)
                            nc.vector.tensor_mul(Ei[:, tt], m1, e2)

                        # context: for each 128-row block of queries
                        for sj in range(SWS // P):
                            st_idx = ss * (SWS // P) + sj
                            qsl = slice(sj * P, (sj + 1) * P)
                            pcA = pctx.tile([P, 2 * DH + 1], FP, tag="pcA")
                            pcB = pctx.tile([P, 2 * DH + 1], FP, tag="pcB")
                            for tt in range(NT):
                                st, sp = tt == 0, tt == NT - 1
                                nc.tensor.matmul(pcA, Er[:, tt, qsl], vaug[h][:, tt],
                                                 start=st, stop=sp)
                                nc.tensor.matmul(pcB, Ei[:, tt, qsl], vaug[h][:, tt],
                                                 start=st, stop=sp)
                            # Act stages the tiny context tiles out of PSUM
                            # (freeing the banks for the next sj) and takes
                            # the reciprocals; the Pool engine does the
                            # complex combine. DVE stays out of the ctx
                            # readout entirely.
                            sA2 = ctxs.tile([P, 2 * DH + 1], FP, tag="sA2")
                            sB2 = ctxs.tile([P, 2 * DH + 1], FP, tag="sB2")
                            nc.scalar.activation(sA2, pcA, AF.Copy)
                            nc.scalar.activation(sB2, pcB, AF.Copy)
                            rr = ctxs.tile([P, 1], FP, tag="rr")
                            ri = ctxs.tile([P, 1], FP, tag="ri")
                            nc.vector.reciprocal(rr, sA2[:, 2 * DH:2 * DH + 1])
                            nc.vector.reciprocal(ri, sB2[:, 2 * DH:2 * DH + 1])
                            # cr = A/sumr - Bvi/sumi ; ci = Avi/sumr + Bvr/sumi
                            tb = ctxs.tile([P, DH], FP, tag="tb")
                            td = ctxs.tile([P, DH], FP, tag="td")
                            cr = ctxs.tile([P, DH], FP, tag="cr")
                            ci = ctxs.tile([P, DH], FP, tag="ci")
                            nc.vector.tensor_scalar_mul(tb, sB2[:, DH:2 * DH], ri)
                            nc.vector.scalar_tensor_tensor(
                                cr, sA2[:, 0:DH], rr, tb, op0=OP.mult,
                                op1=OP.subtract)
                            nc.vector.tensor_scalar_mul(td, sB2[:, 0:DH], ri)
                            nc.vector.scalar_tensor_tensor(
                                ci, sA2[:, DH:2 * DH], rr, td, op0=OP.mult,
                                op1=OP.add)
                            nc.sync.dma_start(out_d[0, h, st_idx], cr)
                            nc.sync.dma_start(out_d[1, h, st_idx], ci)

            ctx_hsp.__exit__(None, None, None)

    if split_waits:
        _split_multi_waits(nc)
    return nc


def _prep_core_inputs(inputs, core):
    b = core // (N_CORES // B)
    hb = core % (N_CORES // B)
    heads = list(range(hb * HPC, (hb + 1) * HPC))
    cols = slice(hb * VC, (hb + 1) * VC)

    f32 = lambda x: np.ascontiguousarray(np.asarray(x, dtype=np.float32))
    hr = f32(inputs["hidden_r"][b]).T    # [D, S]
    hi = f32(inputs["hidden_i"][b]).T

    wv = np.concatenate(
        [f32(inputs["Wv_r"])[:, cols], f32(inputs["Wv_i"])[:, cols],
         f32(inputs["Wcb_r"])[:, heads], f32(inputs["Wcb_i"])[:, heads]], axis=1)
    bv = np.concatenate(
        [np.ones(P, np.float32),
         f32(inputs["bv_r"])[cols], f32(inputs["bv_i"])[cols],
         np.zeros(2 * HPC, np.float32)])

    mr = f32(inputs["mix_r"])[heads]     # [HPC, DK]
    mi = f32(inputs["mix_i"])[heads]
    mixv = np.stack([mr, mi, -mi], axis=-1)  # [HPC, DK, 3]

    c = np.ascontiguousarray
    return {
        "hTr": c(hr.reshape(ND, P, S)),
        "hTi": c(hi.reshape(ND, P, S)),
        "wqr": c(f32(inputs["Wq_r"]).reshape(ND, P, DK)),
        "wqi": c(f32(inputs["Wq_i"]).reshape(ND, P, DK)),
        "wkr": c(f32(inputs["Wk_r"]).reshape(ND, P, DK)),
        "wki": c(f32(inputs["Wk_i"]).reshape(ND, P, DK)),
        "wvcb": c(wv.reshape(ND, P, WVCB)),
        "bvec": c(bv.reshape(1, P + WVCB)),
        "mixv": c(mixv.reshape(HPC, NDK, P, 3).transpose(2, 0, 1, 3)
                  .reshape(P, HPC * NDK * 3)),
    }


def kernel(**inputs):
    global _compiled, LAST_RESULTS
    if _compiled is None:
        _compiled = _build()
    nc = _compiled

    in_maps = [_prep_core_inputs(inputs, c) for c in range(N_CORES)]
    res = run_bass_kernel_spmd(nc, in_maps, core_ids=list(range(N_CORES)),
                               trace=TRACE)
    LAST_RESULTS = res

    out = np.zeros((2, B, S, DV), np.float32)
    for core in range(N_CORES):
        b = core // (N_CORES // B)
        hb = core % (N_CORES // B)
        oc = res.results[core]["out"]  # [2, HPC, NT, P, DH]
        for j in range(HPC):
            h = hb * HPC + j
            out[:, b, :, h * DH:(h + 1) * DH] = oc[:, j].reshape(2, S, DH)
    return out
